# revision 29
# baseline (speedup 1.0000x reference)
"""Multi-head causal attention (B=4, S=2048, D=1024, H=16, dk=dv=64) on 8 NeuronCores.

Sharding: core c -> (batch b = c//2, head-group g = c%2 of 8 heads).
Each core computes Q/K/V projections for its batch restricted to its 8 heads,
causal softmax attention, and a partial output projection with its 512 rows of
Wo.  The host sums the two partials per batch and adds the constant correction
bv @ Wo + bo (bv passes through attention linearly because softmax rows sum
to 1).

v4 highlights (per core):
  - Projections run as compensated-fp8 DoubleRow matmuls: host splits x^T and
    the (range-scaled) weights into fp8 hi+lo pairs; x@W ~ xh@Wh + xh@Wl +
    xl@Wh costs 3 DoubleRow passes = 0.75x the f32r cost (measured end-to-end
    error 0.1%).
  - Q^T/K^T are emitted directly in the DoubleRow-packed fp8 layout
    ([128 = 4 heads x 32 dk, 2 dk-halves, S]) by permuting W's columns on the
    host, so the score matmuls run fp8 DoubleRow at 0.5 cycles/row with no
    repacking.  Scale 16 per side, undone by the exp's 1/256 input scale.
  - exp on ACT (f32 PSUM scores -> fp16 P^T); causal-masked diagonal chunks
    multiplied by a triangular 0/1 mask on the DVE (fp16 4x mode).
  - AV runs in natural orientation (out [128 q, 65]) using all 128 PE
    partitions (2x fewer cycles than transposed) with a ones column for the
    softmax denominator; normalization is a per-partition reciprocal + one
    fused DVE tensor_scalar per head.
  - attn tiles are transposed for the output projection via identity-matmul
    on the PE (128 cycles each).
  - phases are emitted interleaved so ACT exp work overlaps the PE-heavy
    projections and output projection; score tiles are software-pipelined.
"""

import numpy as np
from contextlib import ExitStack

import concourse.bass as bass
import concourse.mybir as mybir
import concourse.tile as tile
from concourse import bacc, bass_utils

N_HEAD, D_MODEL, D_K, D_V = 16, 1024, 64, 64
BATCH, SEQ = 4, 2048
NCORES = 8
S = SEQ
DM = D_MODEL
HV = 8 * D_V          # 512 local head-value columns per core
KC2 = DM // 256       # 4 DoubleRow contraction chunks
NPAIR = 4             # local head pairs
NQT = S // 512        # 4 q-tiles
F32 = mybir.dt.float32
F32R = mybir.dt.float32r
FP16 = mybir.dt.float16
FP8 = mybir.dt.float8e4
DR = mybir.MatmulPerfMode.DoubleRow

_CACHED_NC = None


def _build_nc(nbody=1):
    nc = bacc.Bacc("TRN2", target_bir_lowering=False, debug=False)

    dram = {}
    for nm in ("xh", "xl"):
        dram[nm] = nc.dram_tensor(nm, [DM, S], FP8, kind="ExternalInput").ap()
    for nm in ("wqh", "wql", "wkh", "wkl", "wvh", "wvl"):
        dram[nm] = nc.dram_tensor(nm, [DM, HV], FP8, kind="ExternalInput").ap()
    dram["wo"] = nc.dram_tensor("wo", [HV, DM], FP16, kind="ExternalInput").ap()
    dram["bq"] = nc.dram_tensor("bq", [HV], F32, kind="ExternalInput").ap()
    dram["bk"] = nc.dram_tensor("bk", [HV], F32, kind="ExternalInput").ap()
    dram["masks"] = nc.dram_tensor("masks", [128, 128], FP16, kind="ExternalInput").ap()
    dram["ident"] = nc.dram_tensor("ident", [128, 128], FP16, kind="ExternalInput").ap()
    o = nc.dram_tensor("o", [S, DM], F32, kind="ExternalOutput").ap()

    with tile.TileContext(nc) as tc:
        for _ in range(nbody):
            _build_kernel(tc, nc, dram, o, debug=nbody == -1)
    nc.compile()
    return nc


def _build_debug_nc():
    nc = bacc.Bacc("TRN2", target_bir_lowering=False, debug=False)
    dram = {}
    for nm in ("xh", "xl"):
        dram[nm] = nc.dram_tensor(nm, [DM, S], FP8, kind="ExternalInput").ap()
    for nm in ("wqh", "wql", "wkh", "wkl", "wvh", "wvl"):
        dram[nm] = nc.dram_tensor(nm, [DM, HV], FP8, kind="ExternalInput").ap()
    dram["wo"] = nc.dram_tensor("wo", [HV, DM], FP16, kind="ExternalInput").ap()
    dram["bq"] = nc.dram_tensor("bq", [HV], F32, kind="ExternalInput").ap()
    dram["bk"] = nc.dram_tensor("bk", [HV], F32, kind="ExternalInput").ap()
    dram["masks"] = nc.dram_tensor("masks", [128, 128], FP16, kind="ExternalInput").ap()
    dram["ident"] = nc.dram_tensor("ident", [128, 128], FP16, kind="ExternalInput").ap()
    o = nc.dram_tensor("o", [S, DM], F32, kind="ExternalOutput").ap()
    dbg = {
        "d_qt8": nc.dram_tensor("d_qt8", [64, 2, S], FP8, kind="ExternalOutput").ap(),
        "d_kt8": nc.dram_tensor("d_kt8", [64, 2, S], FP8, kind="ExternalOutput").ap(),
        "d_vpr": nc.dram_tensor("d_vpr", [128, 8 * 65], FP16, kind="ExternalOutput").ap(),
        "d_an": nc.dram_tensor("d_an", [128, HV], FP16, kind="ExternalOutput").ap(),
        "d_an5": nc.dram_tensor("d_an5", [128, HV], FP16, kind="ExternalOutput").ap(),
        "d_an15": nc.dram_tensor("d_an15", [128, HV], FP16, kind="ExternalOutput").ap(),
        "d_at": nc.dram_tensor("d_at", [128, S], FP16, kind="ExternalOutput").ap(),
    }
    with tile.TileContext(nc) as tc:
        _build_kernel(tc, nc, dram, o, debug=dbg)
    nc.compile()
    return nc


def _build_kernel(tc, nc, dram, o, debug=None):
    EXP = mybir.ActivationFunctionType.Exp
    MULT = mybir.AluOpType.mult
    ADD = mybir.AluOpType.add

    with ExitStack() as ctx:
        # ---- persistent tensors (live across phases) ----
        pp = ctx.enter_context(tc.tile_pool(name="persist", bufs=1))
        # packed fp8 Q^T/K^T: pair tile pr holds heads 2pr (base 0) and
        # 2pr+1 (base 32); free dims = (dk-half t, s).  64-partition tiles
        # because AP slices may only start at partition 0/32/64.
        qt8 = [pp.tile([64, 2, S], FP8, name=f"qt8_{q}", tag=f"qt8_{q}") for q in range(4)]
        kt8 = [pp.tile([64, 2, S], FP8, name=f"kt8_{q}", tag=f"kt8_{q}") for q in range(4)]
        vpr = [
            pp.tile([128, 8 * 65], FP16, name=f"vp{sc}", tag=f"vp{sc}")
            for sc in range(S // 128)
        ]
        at_nat = [
            pp.tile([128, HV], FP16, name=f"an{sc}", tag=f"an{sc}")
            for sc in range(S // 128)
        ]
        at_sb = [
            pp.tile([128, S], FP16, name=f"at{p}", tag=f"at{p}")
            for p in range(NPAIR)
        ]
        wo_sb = pp.tile([128, NPAIR * DM], FP16, name="wo_sb", tag="wo_sb")
        mask_sb = pp.tile([128, 128], FP16, name="mask_sb", tag="mask_sb")
        ident_sb = pp.tile([128, 128], FP16, name="ident_sb", tag="ident_sb")
        bq_sb = pp.tile([128, NPAIR], F32, name="bq_sb", tag="bq_sb")
        bk_sb = pp.tile([128, NPAIR], F32, name="bk_sb", tag="bk_sb")
        # zero row for the au-zeroing matmul (see b_head)
        z_sb = pp.tile([1, 4 * 65], FP16, name="z_sb", tag="z_sb")
        # weights: per tensor a [128, KC2 * 2 * 512] fp8 tile, chunk kc2 at
        # [:, kc2, t, :]
        pa = ctx.enter_context(tc.tile_pool(name="pa", bufs=1))
        w_sb = {
            nm: pa.tile([128, KC2, 2, HV], FP8, name=f"{nm}_sb", tag=f"{nm}_sb")
            for nm in ("wqh", "wql", "wkh", "wkl", "wvh", "wvl")
        }
        pax = ctx.enter_context(tc.tile_pool(name="pa_x", bufs=10))
        pb = ctx.enter_context(tc.tile_pool(name="pb", bufs=4))
        pbr = ctx.enter_context(tc.tile_pool(name="pb_r", bufs=4))
        pc_pool = ctx.enter_context(tc.tile_pool(name="pc", bufs=3))
        # PSUM: pj 2x1 + st 2x2 + au 2x1 = 8 banks.
        psum = ctx.enter_context(tc.tile_pool(name="psum", bufs=2, space="PSUM"))

        nc.gpsimd.memset(z_sb[:], 0.0)
        for sc in range(S // 128):
            nc.gpsimd.memset(vpr[sc][:], 1.0)

        SH = S // 2

        # ---------- DMA ----------
        xts = {}

        def load_x(half):
            s0 = half * SH
            for kc2 in range(KC2):
                for v in ("xh", "xl"):
                    t = pax.tile([128, 2, SH], FP8, name=f"xt_{v}_{half}_{kc2}", tag="xt")
                    nc.sync.dma_start(
                        out=t[:],
                        in_=dram[v][kc2 * 256 : (kc2 + 1) * 256, s0 : s0 + SH].rearrange(
                            "(t p) s -> p t s", p=128
                        ),
                    )
                    xts[(v, half, kc2)] = t

        def load_w(nm):
            for kc2 in range(KC2):
                nc.sync.dma_start(
                    out=w_sb[nm][:, kc2],
                    in_=dram[nm][kc2 * 256 : (kc2 + 1) * 256, :].rearrange(
                        "(t p) c -> p t c", p=128
                    ),
                )

        # x(half 0) and wq/wk interleaved so the first Q/K projections (the
        # gate for the first exp) run right behind the DMA stream; wv/V'
        # follow (first needed by the first AV, a few us later).
        for kc2 in range(KC2):
            for v in ("xh", "xl"):
                t = pax.tile([128, 2, SH], FP8, name=f"xt_{v}_0_{kc2}", tag="xt")
                nc.sync.dma_start(
                    out=t[:],
                    in_=dram[v][kc2 * 256 : (kc2 + 1) * 256, 0:SH].rearrange(
                        "(t p) s -> p t s", p=128
                    ),
                )
                xts[(v, 0, kc2)] = t
            for nm in ("wqh", "wql", "wkh", "wkl"):
                nc.sync.dma_start(
                    out=w_sb[nm][:, kc2],
                    in_=dram[nm][kc2 * 256 : (kc2 + 1) * 256, :].rearrange(
                        "(t p) c -> p t c", p=128
                    ),
                )
        nc.sync.dma_start(
            out=bq_sb[:], in_=dram["bq"].rearrange("(bl r) -> r bl", r=128)
        )
        nc.sync.dma_start(
            out=bk_sb[:], in_=dram["bk"].rearrange("(bl r) -> r bl", r=128)
        )
        load_w("wvh")
        load_w("wvl")
        nc.sync.dma_start(out=mask_sb[:], in_=dram["masks"])
        nc.sync.dma_start(out=ident_sb[:], in_=dram["ident"])

        # compensated-fp8 product passes: (xh,Wh), (xh,Wl), (xl,Wh)
        COMB = (("xh", "h"), ("xh", "l"), ("xl", "h"))

        # ---------- phase A building blocks ----------
        def a_v_chunk(half, ss):
            """V' tile for s-chunk (half*8 + ss): out [128 s, 512 cols]."""
            sc = half * (SH // 128) + ss
            vp_ps = psum.tile([128, 512], F32, name=f"vps_{sc}", tag="pj")
            n = 0
            for kc2 in range(KC2):
                for xv, wv_ in COMB:
                    n += 1
                    nc.tensor.matmul(
                        vp_ps[:],
                        lhsT=xts[(xv, half, kc2)][:, :, ss * 128 : (ss + 1) * 128],
                        rhs=w_sb["wv" + wv_][:, kc2],
                        start=(n == 1),
                        stop=(n == 3 * KC2),
                        perf_mode=DR,
                    )
            nc.vector.tensor_scalar(
                out=vpr[sc][:].rearrange("p (h c) -> p h c", h=8)[:, :, 0:64],
                in0=vp_ps[:].rearrange("p (h c) -> p h c", h=8),
                scalar1=1.0 / 32.0,
                scalar2=None,
                op0=MULT,
            )

        def a_qk_block(wch, dst, b_sb, s2, half, bl, nt):
            """One packed-fp8 projection block: psum [128 cols', 512 s] ->
            fp8 quad tile. bl = quad*2 + t."""
            quad, tt = divmod(bl, 2)
            s0 = half * SH
            qs = s0 + nt * 512
            ps = psum.tile([128, 512], F32, name=f"qk_{wch}_{bl}_{qs}", tag="pj")
            n = 0
            for kc2 in range(KC2):
                for xv, wv_ in COMB:
                    n += 1
                    nc.tensor.matmul(
                        ps[:],
                        lhsT=w_sb[wch + wv_][:, kc2, :, bl * 128 : (bl + 1) * 128],
                        rhs=xts[(xv, half, kc2)][:, :, nt * 512 : (nt + 1) * 512],
                        start=(n == 1),
                        stop=(n == 3 * KC2),
                        perf_mode=DR,
                    )
            # evacuate 4 heads into two pair tiles; the upper psum half
            # partition-shifts down to the pair tile's base.
            for half_ps in range(2):
                nc.vector.tensor_scalar(
                    out=dst[2 * quad + half_ps][:, tt, qs : qs + 512],
                    in0=ps[half_ps * 64 : (half_ps + 1) * 64, :],
                    scalar1=b_sb[half_ps * 64 : (half_ps + 1) * 64, bl : bl + 1],
                    scalar2=s2,
                    op0=ADD,
                    op1=MULT,
                )

        # ---------- phase B building block ----------
        def b_head(h, j):
            """Attention for head h, q-tile j (512 queries)."""
            pr, hq = divmod(h, 2)
            r32 = hq * 32
            nk = 4 * j + 4  # causal: k-chunks 0..nk-1
            # au: 4 q-subchunks side by side, each [128 q, 64 attn + 1 denom].
            # The 4 causal accumulation groups share one PSUM bank, and a
            # start=True matmul marks the WHOLE 2KB bank pending-zero (which
            # would wipe sibling groups' partial sums) — so zero the tile with
            # one spanning matmul and accumulate everything with start=False.
            au = psum.tile([128, 4 * 65], F32, name=f"au_{h}_{j}", tag="au")
            nc.tensor.matmul(
                au[:],
                lhsT=z_sb[0:1, 0:128],
                rhs=z_sb[0:1, :],
                start=True,
                stop=True,
                skip_group_check=True,
            )

            def mk_st(pc):
                vp = max(0, 128 * (2 * pc) - 512 * j)
                st = psum.tile([128, 1024], F32, name=f"st_{h}_{j}_{pc}", tag="st")
                for u in range(2):
                    kc = 2 * pc + u
                    nc.tensor.matmul(
                        st[:, u * 512 + vp : (u + 1) * 512],
                        lhsT=kt8[pr][r32 : r32 + 32, :, kc * 128 : (kc + 1) * 128],
                        rhs=qt8[pr][r32 : r32 + 32, :, j * 512 + vp : (j + 1) * 512],
                        start=True,
                        stop=True,
                        perf_mode=DR,
                    )
                pt = pb.tile([128, 1024], FP16, name=f"pt_{h}_{j}_{pc}", tag="pt")
                st3 = st[:].rearrange("p (u c) -> p u c", u=2)
                pt3 = pt[:].rearrange("p (u c) -> p u c", u=2)
                nc.scalar.activation(
                    pt3[:, :, vp:512], st3[:, :, vp:512], EXP, scale=1.0 / 256.0
                )
                for u in range(2):
                    kc = 2 * pc + u
                    i = kc - 4 * j
                    if i >= 0:  # diagonal chunk: triangular 0/1 mask (on the
                        # otherwise-idle Pool engine, off the DVE queue)
                        c0 = u * 512 + 128 * i
                        nc.gpsimd.tensor_tensor(
                            out=pt[:, c0 : c0 + 128],
                            in0=pt[:, c0 : c0 + 128],
                            in1=mask_sb[:, 0:128],
                            op=MULT,
                        )
                return pt

            def mk_av(pc, pt):
                for u in range(2):
                    kc = 2 * pc + u
                    for qs4 in range(4):
                        jq = 4 * j + qs4
                        if kc > jq:
                            continue
                        nc.tensor.matmul(
                            au[:, qs4 * 65 : qs4 * 65 + 65],
                            lhsT=pt[:, u * 512 + qs4 * 128 : u * 512 + (qs4 + 1) * 128],
                            rhs=vpr[kc][:, h * 65 : (h + 1) * 65],
                            start=False,
                            stop=(kc == jq),
                            skip_group_check=True,
                        )

            # software pipeline: issue st(pc+1) before AV(pc) so the PE keeps
            # running while ACT computes exp(pc).
            pts = {0: mk_st(0)}
            for pc in range(nk // 2):
                if pc + 1 < nk // 2:
                    pts[pc + 1] = mk_st(pc + 1)
                mk_av(pc, pts.pop(pc))

            # normalization: per-partition reciprocal of the 4 denominator
            # columns, then one fused multiply per q-subchunk.
            rcp = pbr.tile([128, 4], F32R, name=f"r_{h}_{j}", tag="r")
            with nc.allow_low_precision(
                reason="f32r output is bit-identical to f32 here"
            ):
                nc.vector.reciprocal(
                    out=rcp[:],
                    in_=au[:].rearrange("p (q c) -> p q c", c=65)[:, :, 64],
                )
            for qs4 in range(4):
                nc.vector.tensor_scalar(
                    out=at_nat[4 * j + qs4][:, h * 64 : (h + 1) * 64],
                    in0=au[:, qs4 * 65 : qs4 * 65 + 64],
                    scalar1=rcp[:, qs4 : qs4 + 1].bitcast(F32),
                    scalar2=None,
                    op0=MULT,
                )

        # ---------- transpose + phase C building blocks ----------
        def t_block(j, p):
            """Transpose at_nat[4j..4j+3] columns of pair p into at_sb[p]."""
            tp = psum.tile([128, 512], F32, name=f"tp_{j}_{p}", tag="pj")
            for qs4 in range(4):
                nc.tensor.matmul(
                    tp[:, qs4 * 128 : (qs4 + 1) * 128],
                    lhsT=at_nat[4 * j + qs4][:, p * 128 : (p + 1) * 128],
                    rhs=ident_sb[:],
                    start=True,
                    stop=True,
                )
            nc.vector.tensor_copy(
                out=at_sb[p][:, j * 512 : (j + 1) * 512], in_=tp[:]
            )

        def c_chunk(sc):
            osb = pc_pool.tile([128, DM], F32, name=f"osb_{sc}", tag="osb")
            for m in range(DM // 512):
                op_ps = psum.tile([128, 512], F32, name=f"ops_{sc}_{m}", tag="pj")
                for p in range(NPAIR):
                    nc.tensor.matmul(
                        op_ps[:],
                        lhsT=at_sb[p][:, sc * 128 : (sc + 1) * 128],
                        rhs=wo_sb[:, p * DM + m * 512 : p * DM + (m + 1) * 512],
                        start=(p == 0),
                        stop=(p == NPAIR - 1),
                    )
                nc.vector.tensor_copy(
                    out=osb[:, m * 512 : (m + 1) * 512], in_=op_ps[:]
                )
            nc.sync.dma_start(out=o[sc * 128 : (sc + 1) * 128, :], in_=osb[:])

        # ---------- interleaved schedule ----------
        def qk(half, bl, nt):
            return [
                lambda: a_qk_block("wq", qt8, bq_sb, 1.0 / 16.0, half, bl, nt),
                lambda: a_qk_block("wk", kt8, bk_sb, 0.5, half, bl, nt),
            ]

        # Minimal A prefix for B(h=0..3, j=0): Q/K pair tiles 0/1 (psum
        # blocks bl=0,1 of half 0) and V' chunks 0-3.
        for f in qk(0, 0, 0) + qk(0, 1, 0):
            f()
        for ss in range(4):
            a_v_chunk(0, ss)

        load_x(1)
        # remaining A work spread across B j=0..2 respecting column needs
        a_j0 = (
            qk(0, 2, 0) + qk(0, 3, 0)
            + [lambda ss=ss: a_v_chunk(0, ss) for ss in range(4, 8)]
            + qk(0, 0, 1) + qk(0, 1, 1)
        )
        a_j1 = (
            qk(0, 2, 1) + qk(0, 3, 1)
            + [lambda ss=ss: a_v_chunk(1, ss) for ss in range(4)]
            + qk(1, 0, 0) + qk(1, 1, 0) + qk(1, 2, 0) + qk(1, 3, 0)
        )
        a_j2 = (
            [lambda ss=ss: a_v_chunk(1, ss) for ss in range(4, 8)]
            + qk(1, 0, 1) + qk(1, 1, 1) + qk(1, 2, 1) + qk(1, 3, 1)
        )

        for h in range(8):
            b_head(h, 0)
            a_j0.pop(0)()
            if h % 2 == 1:
                a_j0.pop(0)()
        for p in range(NPAIR):
            t_block(0, p)
        nc.sync.dma_start(
            out=wo_sb[:].rearrange("p (pair c) -> p pair c", pair=NPAIR),
            in_=dram["wo"].rearrange("(pair p) c -> p pair c", p=128),
        )
        for h in range(8):
            b_head(h, 1)
            a_j1.pop(0)()
            a_j1.pop(0)()
        for p in range(NPAIR):
            t_block(1, p)
        for h in range(8):
            b_head(h, 2)
            a_j2.pop(0)()
            if h % 2 == 1:
                a_j2.pop(0)()
            if h % 2 == 1 and h // 2 < 4:
                c_chunk(h // 2)
        for p in range(NPAIR):
            t_block(2, p)
        for h in range(8):
            b_head(h, 3)
            c_chunk(4 + h)
        for p in range(NPAIR):
            t_block(3, p)
        for sc in range(12, 16):
            c_chunk(sc)

        if debug:
            nc.sync.dma_start(out=debug["d_qt8"], in_=qt8[0][:])
            nc.sync.dma_start(out=debug["d_kt8"], in_=kt8[0][:])
            nc.sync.dma_start(out=debug["d_vpr"], in_=vpr[0][:])
            nc.sync.dma_start(out=debug["d_an"], in_=at_nat[0][:])
            nc.sync.dma_start(out=debug["d_an5"], in_=at_nat[5][:])
            nc.sync.dma_start(out=debug["d_an15"], in_=at_nat[15][:])
            nc.sync.dma_start(out=debug["d_at"], in_=at_sb[0][:])


def _masks_np():
    # tri[r, c] = 1 where k_local <= q_local (unmasked on the diagonal block)
    r = np.arange(128)[:, None]
    c = np.arange(128)[None, :]
    return (c >= r).astype(np.float16)


def _qk_perm():
    """Column permutation mapping packed index bl*128 + (h%4)*32 + p to the
    natural column h*64 + t*32 + p (bl = (h//4)*2 + t)."""
    perm = np.empty(HV, np.int64)
    for h in range(8):
        for t in range(2):
            for p in range(32):
                bl = (h // 4) * 2 + t
                perm[bl * 128 + (h % 4) * 32 + p] = h * 64 + t * 32 + p
    return perm


def _split8(a):
    import ml_dtypes

    hi = np.asarray(a, np.float32).astype(ml_dtypes.float8_e4m3)
    lo = (np.asarray(a, np.float32) - hi.astype(np.float32)).astype(
        ml_dtypes.float8_e4m3
    )
    return hi, lo


def make_in_maps(input, Wq, bq, Wk, bk, Wv, Wo):
    scale = np.float32(1.0 / np.sqrt(D_K))
    masks = _masks_np()
    ident = np.eye(128, dtype=np.float16)
    perm = _qk_perm()
    input = np.asarray(input, np.float32)
    in_maps = []
    for c in range(NCORES):
        b, g = divmod(c, 2)
        cols = slice(g * HV, (g + 1) * HV)
        xh, xl = _split8(input[b].T)
        wqh, wql = _split8(np.asarray(Wq, np.float32)[:, cols][:, perm] * (scale * 256))
        wkh, wkl = _split8(np.asarray(Wk, np.float32)[:, cols][:, perm] * 32)
        wvh, wvl = _split8(np.asarray(Wv, np.float32)[:, cols] * 32)
        in_maps.append(
            {
                "xh": np.ascontiguousarray(xh),
                "xl": np.ascontiguousarray(xl),
                "wqh": np.ascontiguousarray(wqh),
                "wql": np.ascontiguousarray(wql),
                "wkh": np.ascontiguousarray(wkh),
                "wkl": np.ascontiguousarray(wkl),
                "wvh": np.ascontiguousarray(wvh),
                "wvl": np.ascontiguousarray(wvl),
                "bq": np.ascontiguousarray(
                    np.asarray(bq, np.float32)[cols][perm] * (scale * 256)
                ),
                "bk": np.ascontiguousarray(np.asarray(bk, np.float32)[cols][perm] * 32),
                "wo": np.ascontiguousarray(
                    np.asarray(Wo, np.float32)[g * HV : (g + 1) * HV, :]
                ).astype(np.float16),
                "masks": masks,
                "ident": ident,
            }
        )
    return in_maps


def _numpy_fallback(input, attn_mask, Wq, bq, Wk, bk, Wv, bv, Wo, bo):
    """Host fallback for non-causal masks (should not trigger in practice)."""
    x = np.asarray(input, np.float32)
    mask = np.asarray(attn_mask)
    B, S_, _ = x.shape
    scale = np.float32(1.0 / np.sqrt(D_K))
    out = np.empty((B, S_, D_MODEL), np.float32)
    for b in range(B):
        q = (x[b] @ Wq + bq).reshape(S_, N_HEAD, D_K)
        k = (x[b] @ Wk + bk).reshape(S_, N_HEAD, D_K)
        v = (x[b] @ Wv + bv).reshape(S_, N_HEAD, D_V)
        attn = np.empty((S_, N_HEAD, D_V), np.float32)
        for h in range(N_HEAD):
            score = (q[:, h] @ k[:, h].T) * scale
            score = np.where(mask, -np.inf, score)
            score -= score.max(axis=-1, keepdims=True)
            p = np.exp(score)
            p /= p.sum(axis=-1, keepdims=True)
            attn[:, h] = p @ v[:, h]
        out[b] = attn.reshape(S_, N_HEAD * D_V) @ Wo + bo
    return out


_CACHED_RUNNER = None


def _make_runner(nc):
    """Build the shard_map-jitted PJRT executor once; reuse across calls."""
    import jax
    from jax.sharding import Mesh, PartitionSpec
    from jax.experimental.shard_map import shard_map
    from concourse import bass2jax

    bass2jax.install_neuronx_cc_hook()
    partition_name = nc.partition_id_tensor.name if nc.partition_id_tensor else None
    in_names, out_names, out_avals, zero_outs = [], [], [], []
    for alloc in nc.m.functions[0].allocations:
        if not isinstance(alloc, mybir.MemoryLocationSet):
            continue
        name = alloc.memorylocations[0].name
        if alloc.kind == "ExternalInput":
            if name != partition_name:
                in_names.append(name)
        elif alloc.kind == "ExternalOutput":
            out_names.append(name)
            shape = tuple(alloc.tensor_shape)
            dtype = mybir.dt.np(alloc.dtype)
            out_avals.append(jax.core.ShapedArray(shape, dtype))
            zero_outs.append(np.zeros(shape, dtype))
    n_params = len(in_names)
    n_outs = len(out_avals)
    all_in_names = list(in_names) + list(out_names)
    if partition_name is not None:
        all_in_names.append(partition_name)

    def _body(*args):
        operands = list(args)
        if partition_name is not None:
            operands.append(bass2jax.partition_id_tensor())
        outs = bass2jax._bass_exec_p.bind(
            *operands,
            out_avals=tuple(out_avals),
            in_names=tuple(all_in_names),
            out_names=tuple(out_names),
            lowering_input_output_aliases=(),
            sim_require_finite=True,
            sim_require_nnan=True,
            nc=nc,
        )
        return tuple(outs)

    devices = jax.devices()[:NCORES]
    mesh = Mesh(np.asarray(devices), ("core",))
    sharded = jax.jit(
        shard_map(
            _body,
            mesh=mesh,
            in_specs=(PartitionSpec("core"),) * (n_params + n_outs),
            out_specs=(PartitionSpec("core"),) * n_outs,
            check_rep=False,
        ),
        donate_argnums=tuple(range(n_params, n_params + n_outs)),
        keep_unused=True,
    )

    def run(in_maps):
        concat_in = [
            np.concatenate(
                [np.asarray(in_maps[c][nm]) for c in range(NCORES)], axis=0
            )
            for nm in in_names
        ]
        concat_zeros = [
            np.zeros((NCORES * z.shape[0], *z.shape[1:]), z.dtype) for z in zero_outs
        ]
        out_arrs = sharded(*concat_in, *concat_zeros)
        return [
            {
                nm: np.asarray(out_arrs[i]).reshape(NCORES, *out_avals[i].shape)[c]
                for i, nm in enumerate(out_names)
            }
            for c in range(NCORES)
        ]

    return run


def kernel(input, attn_mask, Wq, bq, Wk, bk, Wv, bv, Wo, bo):
    causal = np.triu(np.ones((SEQ, SEQ), bool), k=1)
    if not np.array_equal(np.asarray(attn_mask), causal):
        return _numpy_fallback(input, attn_mask, Wq, bq, Wk, bk, Wv, bv, Wo, bo)

    global _CACHED_NC, _CACHED_RUNNER
    if _CACHED_NC is None:
        _CACHED_NC = _build_nc()

    in_maps = make_in_maps(input, Wq, bq, Wk, bk, Wv, Wo)
    try:
        if _CACHED_RUNNER is None:
            _CACHED_RUNNER = _make_runner(_CACHED_NC)
        outs = _CACHED_RUNNER(in_maps)
    except Exception:
        # jit-caching fast path failed (e.g. jax version skew) — use the
        # stock executor.
        _CACHED_RUNNER = None
        outs = bass_utils.run_bass_kernel_spmd(
            _CACHED_NC, in_maps, core_ids=list(range(NCORES))
        ).results

    corr = (
        np.asarray(bv, np.float32) @ np.asarray(Wo, np.float32)
        + np.asarray(bo, np.float32)
    ).astype(np.float32)
    out = np.empty((BATCH, SEQ, D_MODEL), np.float32)
    for b in range(BATCH):
        out[b] = outs[2 * b]["o"] + outs[2 * b + 1]["o"] + corr[None, :]
    return out


# revision 32
# speedup vs baseline: 1.0847x; 1.0847x over previous
"""Multi-head causal attention (B=4, S=2048, D=1024, H=16, dk=dv=64) on 8 NeuronCores.

Sharding: core c -> (batch b = c//2, head-group g = c%2 of 8 heads).
Each core computes Q/K/V projections for its batch restricted to its 8 heads,
causal softmax attention, and a partial output projection with its 512 rows of
Wo.  The host sums the two partials per batch and adds the constant correction
bv @ Wo + bo (bv passes through attention linearly because softmax rows sum
to 1).

v4 highlights (per core):
  - Projections run as compensated-fp8 DoubleRow matmuls: host splits x^T and
    the (range-scaled) weights into fp8 hi+lo pairs; x@W ~ xh@Wh + xh@Wl +
    xl@Wh costs 3 DoubleRow passes = 0.75x the f32r cost (measured end-to-end
    error 0.1%).
  - Q^T/K^T are emitted directly in the DoubleRow-packed fp8 layout
    ([128 = 4 heads x 32 dk, 2 dk-halves, S]) by permuting W's columns on the
    host, so the score matmuls run fp8 DoubleRow at 0.5 cycles/row with no
    repacking.  Scale 16 per side, undone by the exp's 1/256 input scale.
  - exp on ACT (f32 PSUM scores -> fp16 P^T); causal-masked diagonal chunks
    multiplied by a triangular 0/1 mask on the DVE (fp16 4x mode).
  - AV runs in natural orientation (out [128 q, 65]) using all 128 PE
    partitions (2x fewer cycles than transposed) with a ones column for the
    softmax denominator; normalization is a per-partition reciprocal + one
    fused DVE tensor_scalar per head.
  - attn tiles are transposed for the output projection via identity-matmul
    on the PE (128 cycles each).
  - phases are emitted interleaved so ACT exp work overlaps the PE-heavy
    projections and output projection; score tiles are software-pipelined.
"""

import numpy as np
from contextlib import ExitStack

import concourse.bass as bass
import concourse.mybir as mybir
import concourse.tile as tile
from concourse import bacc, bass_utils

N_HEAD, D_MODEL, D_K, D_V = 16, 1024, 64, 64
BATCH, SEQ = 4, 2048
NCORES = 8
S = SEQ
DM = D_MODEL
HV = 8 * D_V          # 512 local head-value columns per core
KC2 = DM // 256       # 4 DoubleRow contraction chunks
NPAIR = 4             # local head pairs
NQT = S // 512        # 4 q-tiles
F32 = mybir.dt.float32
F32R = mybir.dt.float32r
FP16 = mybir.dt.float16
FP8 = mybir.dt.float8e4
DR = mybir.MatmulPerfMode.DoubleRow

_CACHED_NC = None


def _build_nc(nbody=1):
    nc = bacc.Bacc("TRN2", target_bir_lowering=False, debug=False)

    dram = {}
    for nm in ("xh", "xl"):
        dram[nm] = nc.dram_tensor(nm, [DM, S], FP8, kind="ExternalInput").ap()
    for nm in ("wqh", "wql", "wkh", "wkl", "wvh", "wvl"):
        dram[nm] = nc.dram_tensor(nm, [DM, HV], FP8, kind="ExternalInput").ap()
    dram["wo"] = nc.dram_tensor("wo", [HV, DM], FP16, kind="ExternalInput").ap()
    dram["bq"] = nc.dram_tensor("bq", [HV], F32, kind="ExternalInput").ap()
    dram["bk"] = nc.dram_tensor("bk", [HV], F32, kind="ExternalInput").ap()
    dram["masks"] = nc.dram_tensor("masks", [128, 128], FP16, kind="ExternalInput").ap()
    dram["ident"] = nc.dram_tensor("ident", [128, 128], FP16, kind="ExternalInput").ap()
    o = nc.dram_tensor("o", [S, DM], F32, kind="ExternalOutput").ap()

    with tile.TileContext(nc) as tc:
        for _ in range(nbody):
            _build_kernel(tc, nc, dram, o, debug=nbody == -1)
    nc.compile()
    return nc


def _build_debug_nc():
    nc = bacc.Bacc("TRN2", target_bir_lowering=False, debug=False)
    dram = {}
    for nm in ("xh", "xl"):
        dram[nm] = nc.dram_tensor(nm, [DM, S], FP8, kind="ExternalInput").ap()
    for nm in ("wqh", "wql", "wkh", "wkl", "wvh", "wvl"):
        dram[nm] = nc.dram_tensor(nm, [DM, HV], FP8, kind="ExternalInput").ap()
    dram["wo"] = nc.dram_tensor("wo", [HV, DM], FP16, kind="ExternalInput").ap()
    dram["bq"] = nc.dram_tensor("bq", [HV], F32, kind="ExternalInput").ap()
    dram["bk"] = nc.dram_tensor("bk", [HV], F32, kind="ExternalInput").ap()
    dram["masks"] = nc.dram_tensor("masks", [128, 128], FP16, kind="ExternalInput").ap()
    dram["ident"] = nc.dram_tensor("ident", [128, 128], FP16, kind="ExternalInput").ap()
    o = nc.dram_tensor("o", [S, DM], F32, kind="ExternalOutput").ap()
    dbg = {
        "d_qt8": nc.dram_tensor("d_qt8", [64, 2, S], FP8, kind="ExternalOutput").ap(),
        "d_kt8": nc.dram_tensor("d_kt8", [64, 2, S], FP8, kind="ExternalOutput").ap(),
        "d_vpr": nc.dram_tensor("d_vpr", [128, 8 * 65], FP16, kind="ExternalOutput").ap(),
        "d_an": nc.dram_tensor("d_an", [128, HV], FP16, kind="ExternalOutput").ap(),
        "d_an5": nc.dram_tensor("d_an5", [128, HV], FP16, kind="ExternalOutput").ap(),
        "d_an15": nc.dram_tensor("d_an15", [128, HV], FP16, kind="ExternalOutput").ap(),
        "d_at": nc.dram_tensor("d_at", [128, S], FP16, kind="ExternalOutput").ap(),
    }
    with tile.TileContext(nc) as tc:
        _build_kernel(tc, nc, dram, o, debug=dbg)
    nc.compile()
    return nc


def _build_kernel(tc, nc, dram, o, debug=None):
    EXP = mybir.ActivationFunctionType.Exp
    MULT = mybir.AluOpType.mult
    ADD = mybir.AluOpType.add

    with ExitStack() as ctx:
        # ---- persistent tensors (live across phases) ----
        pp = ctx.enter_context(tc.tile_pool(name="persist", bufs=1))
        # packed fp8 Q^T/K^T: pair tile pr holds heads 2pr (base 0) and
        # 2pr+1 (base 32); free dims = (dk-half t, s).  64-partition tiles
        # because AP slices may only start at partition 0/32/64.
        qt8 = [pp.tile([64, 2, S], FP8, name=f"qt8_{q}", tag=f"qt8_{q}") for q in range(4)]
        kt8 = [pp.tile([64, 2, S], FP8, name=f"kt8_{q}", tag=f"kt8_{q}") for q in range(4)]
        vpr = [
            pp.tile([128, 8 * 65], FP16, name=f"vp{sc}", tag=f"vp{sc}")
            for sc in range(S // 128)
        ]
        at_nat = [
            pp.tile([128, HV], FP16, name=f"an{sc}", tag=f"an{sc}")
            for sc in range(S // 128)
        ]
        at_sb = [
            pp.tile([128, S], FP16, name=f"at{p}", tag=f"at{p}")
            for p in range(NPAIR)
        ]
        wo_sb = pp.tile([128, NPAIR * DM], FP16, name="wo_sb", tag="wo_sb")
        mask_sb = pp.tile([128, 128], FP16, name="mask_sb", tag="mask_sb")
        ident_sb = pp.tile([128, 128], FP16, name="ident_sb", tag="ident_sb")
        bq_sb = pp.tile([128, NPAIR], F32, name="bq_sb", tag="bq_sb")
        bk_sb = pp.tile([128, NPAIR], F32, name="bk_sb", tag="bk_sb")
        # zero row for the au-zeroing matmul (see b_head)
        z_sb = pp.tile([1, 4 * 65], FP16, name="z_sb", tag="z_sb")
        # weights: per tensor a [128, KC2 * 2 * 512] fp8 tile, chunk kc2 at
        # [:, kc2, t, :]
        pa = ctx.enter_context(tc.tile_pool(name="pa", bufs=1))
        w_sb = {
            nm: pa.tile([128, KC2, 2, HV], FP8, name=f"{nm}_sb", tag=f"{nm}_sb")
            for nm in ("wqh", "wql", "wkh", "wkl", "wvh", "wvl")
        }
        pax = ctx.enter_context(tc.tile_pool(name="pa_x", bufs=4))
        pb = ctx.enter_context(tc.tile_pool(name="pb", bufs=4))
        pbr = ctx.enter_context(tc.tile_pool(name="pb_r", bufs=4))
        pc_pool = ctx.enter_context(tc.tile_pool(name="pc", bufs=3))
        # PSUM: pj 2x1 + st 2x2 + au 2x1 = 8 banks.
        psum = ctx.enter_context(tc.tile_pool(name="psum", bufs=2, space="PSUM"))

        nc.gpsimd.memset(z_sb[:], 0.0)
        for sc in range(S // 128):
            nc.gpsimd.memset(vpr[sc][:], 1.0)

        SH = S // 2

        # ---------- DMA (few, large transfers: HWDGE dispatch is ~0.6us per
        # DMA, so consolidate each tensor-half into one descriptor) ----------
        xtile = {}

        def load_x(half):
            s0 = half * SH
            for v in ("xh", "xl"):
                t = pax.tile([128, KC2, 2, SH], FP8, name=f"xt_{v}_{half}", tag="xt")
                nc.sync.dma_start(
                    out=t[:],
                    in_=dram[v][:, s0 : s0 + SH].rearrange(
                        "(k t p) s -> p k t s", p=128, t=2
                    ),
                )
                xtile[(v, half)] = t

        def xts(v, half, kc2):
            return xtile[(v, half)][:, kc2]

        def load_w(nm):
            nc.sync.dma_start(
                out=w_sb[nm][:],
                in_=dram[nm].rearrange("(k t p) c -> p k t c", p=128, t=2),
            )

        load_x(0)
        for nm in ("wqh", "wql", "wkh", "wkl"):
            load_w(nm)
        nc.sync.dma_start(
            out=bq_sb[:], in_=dram["bq"].rearrange("(bl r) -> r bl", r=128)
        )
        nc.sync.dma_start(
            out=bk_sb[:], in_=dram["bk"].rearrange("(bl r) -> r bl", r=128)
        )
        load_w("wvh")
        load_w("wvl")
        nc.sync.dma_start(out=mask_sb[:], in_=dram["masks"])
        nc.sync.dma_start(out=ident_sb[:], in_=dram["ident"])

        # compensated-fp8 product passes: (xh,Wh), (xh,Wl), (xl,Wh)
        COMB = (("xh", "h"), ("xh", "l"), ("xl", "h"))

        # ---------- phase A building blocks ----------
        def a_v_chunk(half, ss):
            """V' tile for s-chunk (half*8 + ss): out [128 s, 512 cols]."""
            sc = half * (SH // 128) + ss
            vp_ps = psum.tile([128, 512], F32, name=f"vps_{sc}", tag="pj")
            n = 0
            for kc2 in range(KC2):
                for xv, wv_ in COMB:
                    n += 1
                    nc.tensor.matmul(
                        vp_ps[:],
                        lhsT=xts(xv, half, kc2)[:, :, ss * 128 : (ss + 1) * 128],
                        rhs=w_sb["wv" + wv_][:, kc2],
                        start=(n == 1),
                        stop=(n == 3 * KC2),
                        perf_mode=DR,
                    )
            nc.vector.tensor_scalar(
                out=vpr[sc][:].rearrange("p (h c) -> p h c", h=8)[:, :, 0:64],
                in0=vp_ps[:].rearrange("p (h c) -> p h c", h=8),
                scalar1=1.0 / 32.0,
                scalar2=None,
                op0=MULT,
            )

        def a_qk_block(wch, dst, b_sb, s2, half, bl, nt):
            """One packed-fp8 projection block: psum [128 cols', 512 s] ->
            fp8 quad tile. bl = quad*2 + t."""
            quad, tt = divmod(bl, 2)
            s0 = half * SH
            qs = s0 + nt * 512
            ps = psum.tile([128, 512], F32, name=f"qk_{wch}_{bl}_{qs}", tag="pj")
            n = 0
            for kc2 in range(KC2):
                for xv, wv_ in COMB:
                    n += 1
                    nc.tensor.matmul(
                        ps[:],
                        lhsT=w_sb[wch + wv_][:, kc2, :, bl * 128 : (bl + 1) * 128],
                        rhs=xts(xv, half, kc2)[:, :, nt * 512 : (nt + 1) * 512],
                        start=(n == 1),
                        stop=(n == 3 * KC2),
                        perf_mode=DR,
                    )
            # evacuate 4 heads into two pair tiles; the upper psum half
            # partition-shifts down to the pair tile's base.
            for half_ps in range(2):
                nc.vector.tensor_scalar(
                    out=dst[2 * quad + half_ps][:, tt, qs : qs + 512],
                    in0=ps[half_ps * 64 : (half_ps + 1) * 64, :],
                    scalar1=b_sb[half_ps * 64 : (half_ps + 1) * 64, bl : bl + 1],
                    scalar2=s2,
                    op0=ADD,
                    op1=MULT,
                )

        # ---------- phase B building block ----------
        def b_head(h, j):
            """Attention for head h, q-tile j (512 queries)."""
            pr, hq = divmod(h, 2)
            r32 = hq * 32
            nk = 4 * j + 4  # causal: k-chunks 0..nk-1
            # au: 4 q-subchunks side by side, each [128 q, 64 attn + 1 denom].
            # The 4 causal accumulation groups share one PSUM bank, and a
            # start=True matmul marks the WHOLE 2KB bank pending-zero (which
            # would wipe sibling groups' partial sums) — so zero the tile with
            # one spanning matmul and accumulate everything with start=False.
            au = psum.tile([128, 4 * 65], F32, name=f"au_{h}_{j}", tag="au")
            nc.tensor.matmul(
                au[:],
                lhsT=z_sb[0:1, 0:128],
                rhs=z_sb[0:1, :],
                start=True,
                stop=True,
                skip_group_check=True,
            )

            def mk_st(pc):
                vp = max(0, 128 * (2 * pc) - 512 * j)
                st = psum.tile([128, 1024], F32, name=f"st_{h}_{j}_{pc}", tag="st")
                for u in range(2):
                    kc = 2 * pc + u
                    nc.tensor.matmul(
                        st[:, u * 512 + vp : (u + 1) * 512],
                        lhsT=kt8[pr][r32 : r32 + 32, :, kc * 128 : (kc + 1) * 128],
                        rhs=qt8[pr][r32 : r32 + 32, :, j * 512 + vp : (j + 1) * 512],
                        start=True,
                        stop=True,
                        perf_mode=DR,
                    )
                pt = pb.tile([128, 1024], FP16, name=f"pt_{h}_{j}_{pc}", tag="pt")
                st3 = st[:].rearrange("p (u c) -> p u c", u=2)
                pt3 = pt[:].rearrange("p (u c) -> p u c", u=2)
                nc.scalar.activation(
                    pt3[:, :, vp:512], st3[:, :, vp:512], EXP, scale=1.0 / 256.0
                )
                for u in range(2):
                    kc = 2 * pc + u
                    i = kc - 4 * j
                    if i >= 0:  # diagonal chunk: triangular 0/1 mask
                        c0 = u * 512 + 128 * i
                        nc.vector.tensor_tensor(
                            out=pt[:, c0 : c0 + 128],
                            in0=pt[:, c0 : c0 + 128],
                            in1=mask_sb[:, 0:128],
                            op=MULT,
                        )
                return pt

            def mk_av(pc, pt):
                for u in range(2):
                    kc = 2 * pc + u
                    for qs4 in range(4):
                        jq = 4 * j + qs4
                        if kc > jq:
                            continue
                        nc.tensor.matmul(
                            au[:, qs4 * 65 : qs4 * 65 + 65],
                            lhsT=pt[:, u * 512 + qs4 * 128 : u * 512 + (qs4 + 1) * 128],
                            rhs=vpr[kc][:, h * 65 : (h + 1) * 65],
                            start=False,
                            stop=(kc == jq),
                            skip_group_check=True,
                        )

            # software pipeline: issue st(pc+1) before AV(pc) so the PE keeps
            # running while ACT computes exp(pc).
            pts = {0: mk_st(0)}
            for pc in range(nk // 2):
                if pc + 1 < nk // 2:
                    pts[pc + 1] = mk_st(pc + 1)
                mk_av(pc, pts.pop(pc))

            # normalization: per-partition reciprocal of the 4 denominator
            # columns, then one fused multiply per q-subchunk.
            rcp = pbr.tile([128, 4], F32R, name=f"r_{h}_{j}", tag="r")
            with nc.allow_low_precision(
                reason="f32r output is bit-identical to f32 here"
            ):
                nc.vector.reciprocal(
                    out=rcp[:],
                    in_=au[:].rearrange("p (q c) -> p q c", c=65)[:, :, 64],
                )
            for qs4 in range(4):
                nc.vector.tensor_scalar(
                    out=at_nat[4 * j + qs4][:, h * 64 : (h + 1) * 64],
                    in0=au[:, qs4 * 65 : qs4 * 65 + 64],
                    scalar1=rcp[:, qs4 : qs4 + 1].bitcast(F32),
                    scalar2=None,
                    op0=MULT,
                )

        # ---------- transpose + phase C building blocks ----------
        def t_block(j, p):
            """Transpose at_nat[4j..4j+3] columns of pair p into at_sb[p]."""
            tp = psum.tile([128, 512], F32, name=f"tp_{j}_{p}", tag="pj")
            for qs4 in range(4):
                nc.tensor.matmul(
                    tp[:, qs4 * 128 : (qs4 + 1) * 128],
                    lhsT=at_nat[4 * j + qs4][:, p * 128 : (p + 1) * 128],
                    rhs=ident_sb[:],
                    start=True,
                    stop=True,
                )
            nc.vector.tensor_copy(
                out=at_sb[p][:, j * 512 : (j + 1) * 512], in_=tp[:]
            )

        def c_chunk(sc):
            osb = pc_pool.tile([128, DM], F32, name=f"osb_{sc}", tag="osb")
            for m in range(DM // 512):
                op_ps = psum.tile([128, 512], F32, name=f"ops_{sc}_{m}", tag="pj")
                for p in range(NPAIR):
                    nc.tensor.matmul(
                        op_ps[:],
                        lhsT=at_sb[p][:, sc * 128 : (sc + 1) * 128],
                        rhs=wo_sb[:, p * DM + m * 512 : p * DM + (m + 1) * 512],
                        start=(p == 0),
                        stop=(p == NPAIR - 1),
                    )
                nc.vector.tensor_copy(
                    out=osb[:, m * 512 : (m + 1) * 512], in_=op_ps[:]
                )
            nc.sync.dma_start(out=o[sc * 128 : (sc + 1) * 128, :], in_=osb[:])

        # ---------- interleaved schedule ----------
        def qk(half, bl, nt):
            return [
                lambda: a_qk_block("wq", qt8, bq_sb, 1.0 / 16.0, half, bl, nt),
                lambda: a_qk_block("wk", kt8, bk_sb, 0.5, half, bl, nt),
            ]

        # Minimal A prefix for B(h=0..3, j=0): Q/K pair tiles 0/1 (psum
        # blocks bl=0,1 of half 0) and V' chunks 0-3.
        for f in qk(0, 0, 0) + qk(0, 1, 0):
            f()
        for ss in range(4):
            a_v_chunk(0, ss)

        load_x(1)
        # remaining A work spread across B j=0..2 respecting column needs
        a_j0 = (
            qk(0, 2, 0) + qk(0, 3, 0)
            + [lambda ss=ss: a_v_chunk(0, ss) for ss in range(4, 8)]
            + qk(0, 0, 1) + qk(0, 1, 1)
        )
        a_j1 = (
            qk(0, 2, 1) + qk(0, 3, 1)
            + [lambda ss=ss: a_v_chunk(1, ss) for ss in range(4)]
            + qk(1, 0, 0) + qk(1, 1, 0) + qk(1, 2, 0) + qk(1, 3, 0)
        )
        a_j2 = (
            [lambda ss=ss: a_v_chunk(1, ss) for ss in range(4, 8)]
            + qk(1, 0, 1) + qk(1, 1, 1) + qk(1, 2, 1) + qk(1, 3, 1)
        )

        for h in range(8):
            b_head(h, 0)
            a_j0.pop(0)()
            if h % 2 == 1:
                a_j0.pop(0)()
        for p in range(NPAIR):
            t_block(0, p)
        nc.sync.dma_start(
            out=wo_sb[:].rearrange("p (pair c) -> p pair c", pair=NPAIR),
            in_=dram["wo"].rearrange("(pair p) c -> p pair c", p=128),
        )
        for h in range(8):
            b_head(h, 1)
            a_j1.pop(0)()
            a_j1.pop(0)()
        for p in range(NPAIR):
            t_block(1, p)
        for h in range(8):
            b_head(h, 2)
            a_j2.pop(0)()
            if h % 2 == 1:
                a_j2.pop(0)()
            if h % 2 == 1 and h // 2 < 4:
                c_chunk(h // 2)
        for p in range(NPAIR):
            t_block(2, p)
        for h in range(8):
            b_head(h, 3)
            c_chunk(4 + h)
        for p in range(NPAIR):
            t_block(3, p)
        for sc in range(12, 16):
            c_chunk(sc)

        if debug:
            nc.sync.dma_start(out=debug["d_qt8"], in_=qt8[0][:])
            nc.sync.dma_start(out=debug["d_kt8"], in_=kt8[0][:])
            nc.sync.dma_start(out=debug["d_vpr"], in_=vpr[0][:])
            nc.sync.dma_start(out=debug["d_an"], in_=at_nat[0][:])
            nc.sync.dma_start(out=debug["d_an5"], in_=at_nat[5][:])
            nc.sync.dma_start(out=debug["d_an15"], in_=at_nat[15][:])
            nc.sync.dma_start(out=debug["d_at"], in_=at_sb[0][:])


def _masks_np():
    # tri[r, c] = 1 where k_local <= q_local (unmasked on the diagonal block)
    r = np.arange(128)[:, None]
    c = np.arange(128)[None, :]
    return (c >= r).astype(np.float16)


def _qk_perm():
    """Column permutation mapping packed index bl*128 + (h%4)*32 + p to the
    natural column h*64 + t*32 + p (bl = (h//4)*2 + t)."""
    perm = np.empty(HV, np.int64)
    for h in range(8):
        for t in range(2):
            for p in range(32):
                bl = (h // 4) * 2 + t
                perm[bl * 128 + (h % 4) * 32 + p] = h * 64 + t * 32 + p
    return perm


def _split8(a):
    import ml_dtypes

    hi = np.asarray(a, np.float32).astype(ml_dtypes.float8_e4m3)
    lo = (np.asarray(a, np.float32) - hi.astype(np.float32)).astype(
        ml_dtypes.float8_e4m3
    )
    return hi, lo


def make_in_maps(input, Wq, bq, Wk, bk, Wv, Wo):
    scale = np.float32(1.0 / np.sqrt(D_K))
    masks = _masks_np()
    ident = np.eye(128, dtype=np.float16)
    perm = _qk_perm()
    input = np.asarray(input, np.float32)
    in_maps = []
    for c in range(NCORES):
        b, g = divmod(c, 2)
        cols = slice(g * HV, (g + 1) * HV)
        xh, xl = _split8(input[b].T)
        wqh, wql = _split8(np.asarray(Wq, np.float32)[:, cols][:, perm] * (scale * 256))
        wkh, wkl = _split8(np.asarray(Wk, np.float32)[:, cols][:, perm] * 32)
        wvh, wvl = _split8(np.asarray(Wv, np.float32)[:, cols] * 32)
        in_maps.append(
            {
                "xh": np.ascontiguousarray(xh),
                "xl": np.ascontiguousarray(xl),
                "wqh": np.ascontiguousarray(wqh),
                "wql": np.ascontiguousarray(wql),
                "wkh": np.ascontiguousarray(wkh),
                "wkl": np.ascontiguousarray(wkl),
                "wvh": np.ascontiguousarray(wvh),
                "wvl": np.ascontiguousarray(wvl),
                "bq": np.ascontiguousarray(
                    np.asarray(bq, np.float32)[cols][perm] * (scale * 256)
                ),
                "bk": np.ascontiguousarray(np.asarray(bk, np.float32)[cols][perm] * 32),
                "wo": np.ascontiguousarray(
                    np.asarray(Wo, np.float32)[g * HV : (g + 1) * HV, :]
                ).astype(np.float16),
                "masks": masks,
                "ident": ident,
            }
        )
    return in_maps


def _numpy_fallback(input, attn_mask, Wq, bq, Wk, bk, Wv, bv, Wo, bo):
    """Host fallback for non-causal masks (should not trigger in practice)."""
    x = np.asarray(input, np.float32)
    mask = np.asarray(attn_mask)
    B, S_, _ = x.shape
    scale = np.float32(1.0 / np.sqrt(D_K))
    out = np.empty((B, S_, D_MODEL), np.float32)
    for b in range(B):
        q = (x[b] @ Wq + bq).reshape(S_, N_HEAD, D_K)
        k = (x[b] @ Wk + bk).reshape(S_, N_HEAD, D_K)
        v = (x[b] @ Wv + bv).reshape(S_, N_HEAD, D_V)
        attn = np.empty((S_, N_HEAD, D_V), np.float32)
        for h in range(N_HEAD):
            score = (q[:, h] @ k[:, h].T) * scale
            score = np.where(mask, -np.inf, score)
            score -= score.max(axis=-1, keepdims=True)
            p = np.exp(score)
            p /= p.sum(axis=-1, keepdims=True)
            attn[:, h] = p @ v[:, h]
        out[b] = attn.reshape(S_, N_HEAD * D_V) @ Wo + bo
    return out


_CACHED_RUNNER = None


def _make_runner(nc):
    """Build the shard_map-jitted PJRT executor once; reuse across calls."""
    import jax
    from jax.sharding import Mesh, PartitionSpec
    from jax.experimental.shard_map import shard_map
    from concourse import bass2jax

    bass2jax.install_neuronx_cc_hook()
    partition_name = nc.partition_id_tensor.name if nc.partition_id_tensor else None
    in_names, out_names, out_avals, zero_outs = [], [], [], []
    for alloc in nc.m.functions[0].allocations:
        if not isinstance(alloc, mybir.MemoryLocationSet):
            continue
        name = alloc.memorylocations[0].name
        if alloc.kind == "ExternalInput":
            if name != partition_name:
                in_names.append(name)
        elif alloc.kind == "ExternalOutput":
            out_names.append(name)
            shape = tuple(alloc.tensor_shape)
            dtype = mybir.dt.np(alloc.dtype)
            out_avals.append(jax.core.ShapedArray(shape, dtype))
            zero_outs.append(np.zeros(shape, dtype))
    n_params = len(in_names)
    n_outs = len(out_avals)
    all_in_names = list(in_names) + list(out_names)
    if partition_name is not None:
        all_in_names.append(partition_name)

    def _body(*args):
        operands = list(args)
        if partition_name is not None:
            operands.append(bass2jax.partition_id_tensor())
        outs = bass2jax._bass_exec_p.bind(
            *operands,
            out_avals=tuple(out_avals),
            in_names=tuple(all_in_names),
            out_names=tuple(out_names),
            lowering_input_output_aliases=(),
            sim_require_finite=True,
            sim_require_nnan=True,
            nc=nc,
        )
        return tuple(outs)

    devices = jax.devices()[:NCORES]
    mesh = Mesh(np.asarray(devices), ("core",))
    sharded = jax.jit(
        shard_map(
            _body,
            mesh=mesh,
            in_specs=(PartitionSpec("core"),) * (n_params + n_outs),
            out_specs=(PartitionSpec("core"),) * n_outs,
            check_rep=False,
        ),
        donate_argnums=tuple(range(n_params, n_params + n_outs)),
        keep_unused=True,
    )

    def run(in_maps):
        concat_in = [
            np.concatenate(
                [np.asarray(in_maps[c][nm]) for c in range(NCORES)], axis=0
            )
            for nm in in_names
        ]
        concat_zeros = [
            np.zeros((NCORES * z.shape[0], *z.shape[1:]), z.dtype) for z in zero_outs
        ]
        out_arrs = sharded(*concat_in, *concat_zeros)
        return [
            {
                nm: np.asarray(out_arrs[i]).reshape(NCORES, *out_avals[i].shape)[c]
                for i, nm in enumerate(out_names)
            }
            for c in range(NCORES)
        ]

    return run


def kernel(input, attn_mask, Wq, bq, Wk, bk, Wv, bv, Wo, bo):
    causal = np.triu(np.ones((SEQ, SEQ), bool), k=1)
    if not np.array_equal(np.asarray(attn_mask), causal):
        return _numpy_fallback(input, attn_mask, Wq, bq, Wk, bk, Wv, bv, Wo, bo)

    global _CACHED_NC, _CACHED_RUNNER
    if _CACHED_NC is None:
        _CACHED_NC = _build_nc()

    in_maps = make_in_maps(input, Wq, bq, Wk, bk, Wv, Wo)
    try:
        if _CACHED_RUNNER is None:
            _CACHED_RUNNER = _make_runner(_CACHED_NC)
        outs = _CACHED_RUNNER(in_maps)
    except Exception:
        # jit-caching fast path failed (e.g. jax version skew) — use the
        # stock executor.
        _CACHED_RUNNER = None
        outs = bass_utils.run_bass_kernel_spmd(
            _CACHED_NC, in_maps, core_ids=list(range(NCORES))
        ).results

    corr = (
        np.asarray(bv, np.float32) @ np.asarray(Wo, np.float32)
        + np.asarray(bo, np.float32)
    ).astype(np.float32)
    out = np.empty((BATCH, SEQ, D_MODEL), np.float32)
    for b in range(BATCH):
        out[b] = outs[2 * b]["o"] + outs[2 * b + 1]["o"] + corr[None, :]
    return out


# revision 39
# speedup vs baseline: 1.0986x; 1.0128x over previous
"""Multi-head causal attention (B=4, S=2048, D=1024, H=16, dk=dv=64) on 8 NeuronCores.

Sharding: core c -> (batch b = c//2, head-group g = c%2 of 8 heads).
Each core computes Q/K/V projections for its batch restricted to its 8 heads,
causal softmax attention, and a partial output projection with its 512 rows of
Wo.  The host sums the two partials per batch and adds the constant correction
bv @ Wo + bo (bv passes through attention linearly because softmax rows sum
to 1).

v4 highlights (per core):
  - Projections run as compensated-fp8 DoubleRow matmuls: host splits x^T and
    the (range-scaled) weights into fp8 hi+lo pairs; x@W ~ xh@Wh + xh@Wl +
    xl@Wh costs 3 DoubleRow passes = 0.75x the f32r cost (measured end-to-end
    error 0.1%).
  - Q^T/K^T are emitted directly in the DoubleRow-packed fp8 layout
    ([128 = 4 heads x 32 dk, 2 dk-halves, S]) by permuting W's columns on the
    host, so the score matmuls run fp8 DoubleRow at 0.5 cycles/row with no
    repacking.  Scale 16 per side, undone by the exp's 1/256 input scale.
  - exp on ACT (f32 PSUM scores -> fp16 P^T); causal-masked diagonal chunks
    multiplied by a triangular 0/1 mask on the DVE (fp16 4x mode).
  - AV runs in natural orientation (out [128 q, 65]) using all 128 PE
    partitions (2x fewer cycles than transposed) with a ones column for the
    softmax denominator; normalization is a per-partition reciprocal + one
    fused DVE tensor_scalar per head.
  - attn tiles are transposed for the output projection via identity-matmul
    on the PE (128 cycles each).
  - phases are emitted interleaved so ACT exp work overlaps the PE-heavy
    projections and output projection; score tiles are software-pipelined.
"""

import numpy as np
from contextlib import ExitStack

import concourse.bass as bass
import concourse.mybir as mybir
import concourse.tile as tile
from concourse import bacc, bass_utils

N_HEAD, D_MODEL, D_K, D_V = 16, 1024, 64, 64
BATCH, SEQ = 4, 2048
NCORES = 8
S = SEQ
DM = D_MODEL
HV = 8 * D_V          # 512 local head-value columns per core
KC2 = DM // 256       # 4 DoubleRow contraction chunks
NPAIR = 4             # local head pairs
NQT = S // 512        # 4 q-tiles
F32 = mybir.dt.float32
F32R = mybir.dt.float32r
FP16 = mybir.dt.float16
FP8 = mybir.dt.float8e4
DR = mybir.MatmulPerfMode.DoubleRow

_CACHED_NC = None


def _build_nc(nbody=1):
    nc = bacc.Bacc("TRN2", target_bir_lowering=False, debug=False)

    dram = {}
    for nm in ("xh", "xl"):
        dram[nm] = nc.dram_tensor(nm, [DM, S], FP8, kind="ExternalInput").ap()
    for nm in ("wqh", "wql", "wkh", "wkl", "wvh", "wvl"):
        dram[nm] = nc.dram_tensor(nm, [DM, HV], FP8, kind="ExternalInput").ap()
    dram["wo"] = nc.dram_tensor("wo", [HV, DM], FP16, kind="ExternalInput").ap()
    dram["bq"] = nc.dram_tensor("bq", [HV], F32, kind="ExternalInput").ap()
    dram["bk"] = nc.dram_tensor("bk", [HV], F32, kind="ExternalInput").ap()
    dram["masks"] = nc.dram_tensor("masks", [128, 128], FP16, kind="ExternalInput").ap()
    dram["ident"] = nc.dram_tensor("ident", [128, 128], FP16, kind="ExternalInput").ap()
    o = nc.dram_tensor("o", [S, DM], F32, kind="ExternalOutput").ap()

    with tile.TileContext(nc) as tc:
        for _ in range(nbody):
            _build_kernel(tc, nc, dram, o, debug=nbody == -1)
    nc.compile()
    return nc


def _build_debug_nc():
    nc = bacc.Bacc("TRN2", target_bir_lowering=False, debug=False)
    dram = {}
    for nm in ("xh", "xl"):
        dram[nm] = nc.dram_tensor(nm, [DM, S], FP8, kind="ExternalInput").ap()
    for nm in ("wqh", "wql", "wkh", "wkl", "wvh", "wvl"):
        dram[nm] = nc.dram_tensor(nm, [DM, HV], FP8, kind="ExternalInput").ap()
    dram["wo"] = nc.dram_tensor("wo", [HV, DM], FP16, kind="ExternalInput").ap()
    dram["bq"] = nc.dram_tensor("bq", [HV], F32, kind="ExternalInput").ap()
    dram["bk"] = nc.dram_tensor("bk", [HV], F32, kind="ExternalInput").ap()
    dram["masks"] = nc.dram_tensor("masks", [128, 128], FP16, kind="ExternalInput").ap()
    dram["ident"] = nc.dram_tensor("ident", [128, 128], FP16, kind="ExternalInput").ap()
    o = nc.dram_tensor("o", [S, DM], F32, kind="ExternalOutput").ap()
    dbg = {
        "d_qt8": nc.dram_tensor("d_qt8", [64, 2, S], FP8, kind="ExternalOutput").ap(),
        "d_kt8": nc.dram_tensor("d_kt8", [64, 2, S], FP8, kind="ExternalOutput").ap(),
        "d_vpr": nc.dram_tensor("d_vpr", [128, 8 * 65], FP16, kind="ExternalOutput").ap(),
        "d_an": nc.dram_tensor("d_an", [128, HV], FP16, kind="ExternalOutput").ap(),
        "d_an5": nc.dram_tensor("d_an5", [128, HV], FP16, kind="ExternalOutput").ap(),
        "d_an15": nc.dram_tensor("d_an15", [128, HV], FP16, kind="ExternalOutput").ap(),
        "d_at": nc.dram_tensor("d_at", [128, S], FP16, kind="ExternalOutput").ap(),
    }
    with tile.TileContext(nc) as tc:
        _build_kernel(tc, nc, dram, o, debug=dbg)
    nc.compile()
    return nc


def _build_kernel(tc, nc, dram, o, debug=None):
    EXP = mybir.ActivationFunctionType.Exp
    MULT = mybir.AluOpType.mult
    ADD = mybir.AluOpType.add

    with ExitStack() as ctx:
        # ---- persistent tensors (live across phases) ----
        pp = ctx.enter_context(tc.tile_pool(name="persist", bufs=1))
        # packed fp8 Q^T/K^T: pair tile pr holds heads 2pr (base 0) and
        # 2pr+1 (base 32); free dims = (dk-half t, s).  64-partition tiles
        # because AP slices may only start at partition 0/32/64.
        qt8 = [pp.tile([64, 2, S], FP8, name=f"qt8_{q}", tag=f"qt8_{q}") for q in range(4)]
        kt8 = [pp.tile([64, 2, S], FP8, name=f"kt8_{q}", tag=f"kt8_{q}") for q in range(4)]
        vpr = [
            pp.tile([128, 8 * 65], FP16, name=f"vp{sc}", tag=f"vp{sc}")
            for sc in range(S // 128)
        ]
        at_nat = [
            pp.tile([128, HV], FP16, name=f"an{sc}", tag=f"an{sc}")
            for sc in range(S // 128)
        ]
        at_sb = [
            pp.tile([128, S], FP16, name=f"at{p}", tag=f"at{p}")
            for p in range(NPAIR)
        ]
        wo_sb = pp.tile([128, NPAIR * DM], FP16, name="wo_sb", tag="wo_sb")
        mask_sb = pp.tile([128, 128], FP16, name="mask_sb", tag="mask_sb")
        ident_sb = pp.tile([128, 128], FP16, name="ident_sb", tag="ident_sb")
        bq_sb = pp.tile([128, NPAIR], F32, name="bq_sb", tag="bq_sb")
        bk_sb = pp.tile([128, NPAIR], F32, name="bk_sb", tag="bk_sb")
        # zero row for the au-zeroing matmul (see b_head)
        z_sb = pp.tile([1, 4 * 65], FP16, name="z_sb", tag="z_sb")
        # e^(1/256) base for Pool-engine exp offload (pow(base, st) = exp(st/256))
        e256_sb = pp.tile([128, 1024], F32, name="e256_sb", tag="e256_sb")
        # weights: per tensor a [128, KC2 * 2 * 512] fp8 tile, chunk kc2 at
        # [:, kc2, t, :]
        pa = ctx.enter_context(tc.tile_pool(name="pa", bufs=1))
        w_sb = {
            nm: pa.tile([128, KC2, 2, HV], FP8, name=f"{nm}_sb", tag=f"{nm}_sb")
            for nm in ("wqh", "wql", "wkh", "wkl", "wvh", "wvl")
        }
        pax = ctx.enter_context(tc.tile_pool(name="pa_x", bufs=8))
        pb = ctx.enter_context(tc.tile_pool(name="pb", bufs=8))
        pbr = ctx.enter_context(tc.tile_pool(name="pb_r", bufs=4))
        pc_pool = ctx.enter_context(tc.tile_pool(name="pc", bufs=3))
        # PSUM: pj 2x1 + st 2x2 + au 2x1 = 8 banks.
        psum = ctx.enter_context(tc.tile_pool(name="psum", bufs=2, space="PSUM"))

        nc.gpsimd.memset(z_sb[:], 0.0)
        nc.gpsimd.memset(e256_sb[:], float(np.exp(1.0 / 256.0)))
        for sc in range(S // 128):
            nc.gpsimd.memset(vpr[sc][:], 1.0)

        SH = S // 2

        # ---------- DMA (few, large transfers: HWDGE dispatch is ~0.6us per
        # DMA, so consolidate each tensor-half into one descriptor) ----------
        xtile = {}

        def load_x_q(half, nt):
            """load one 512-column slice of x (hi+lo)."""
            s0 = half * SH + nt * 512
            for v in ("xh", "xl"):
                t = pax.tile([128, KC2, 2, 512], FP8, name=f"xt_{v}_{half}_{nt}", tag="xt")
                nc.sync.dma_start(
                    out=t[:],
                    in_=dram[v][:, s0 : s0 + 512].rearrange(
                        "(k t p) s -> p k t s", p=128, t=2
                    ),
                )
                xtile[(v, half, nt)] = t

        def load_x(half):
            for nt in range(2):
                load_x_q(half, nt)

        def xts(v, half, kc2, nt):
            return xtile[(v, half, nt)][:, kc2]

        def load_w(nm):
            nc.sync.dma_start(
                out=w_sb[nm][:],
                in_=dram[nm].rearrange("(k t p) c -> p k t c", p=128, t=2),
            )

        load_x_q(0, 0)
        for nm in ("wqh", "wql", "wkh", "wkl"):
            load_w(nm)
        nc.sync.dma_start(
            out=bq_sb[:], in_=dram["bq"].rearrange("(bl r) -> r bl", r=128)
        )
        nc.sync.dma_start(
            out=bk_sb[:], in_=dram["bk"].rearrange("(bl r) -> r bl", r=128)
        )
        load_w("wvh")
        load_w("wvl")
        load_x_q(0, 1)
        nc.sync.dma_start(out=mask_sb[:], in_=dram["masks"])
        nc.sync.dma_start(out=ident_sb[:], in_=dram["ident"])

        # compensated-fp8 product passes: (xh,Wh), (xh,Wl), (xl,Wh)
        COMB = (("xh", "h"), ("xh", "l"), ("xl", "h"))

        # ---------- phase A building blocks ----------
        def a_v_chunk(half, ss):
            """V' tile for s-chunk (half*8 + ss): out [128 s, 512 cols]."""
            sc = half * (SH // 128) + ss
            vp_ps = psum.tile([128, 512], F32, name=f"vps_{sc}", tag="pj")
            n = 0
            for kc2 in range(KC2):
                for xv, wv_ in COMB:
                    n += 1
                    nc.tensor.matmul(
                        vp_ps[:],
                        lhsT=xts(xv, half, kc2, ss // 4)[:, :, (ss % 4) * 128 : (ss % 4 + 1) * 128],
                        rhs=w_sb["wv" + wv_][:, kc2],
                        start=(n == 1),
                        stop=(n == 3 * KC2),
                        perf_mode=DR,
                    )
            nc.vector.tensor_scalar(
                out=vpr[sc][:].rearrange("p (h c) -> p h c", h=8)[:, :, 0:64],
                in0=vp_ps[:].rearrange("p (h c) -> p h c", h=8),
                scalar1=1.0 / 32.0,
                scalar2=None,
                op0=MULT,
            )

        def a_qk_block(wch, dst, b_sb, s2, half, bl, nt):
            """One packed-fp8 projection block: psum [128 cols', 512 s] ->
            fp8 quad tile. bl = quad*2 + t."""
            quad, tt = divmod(bl, 2)
            s0 = half * SH
            qs = s0 + nt * 512
            ps = psum.tile([128, 512], F32, name=f"qk_{wch}_{bl}_{qs}", tag="pj")
            n = 0
            for kc2 in range(KC2):
                for xv, wv_ in COMB:
                    n += 1
                    nc.tensor.matmul(
                        ps[:],
                        lhsT=w_sb[wch + wv_][:, kc2, :, bl * 128 : (bl + 1) * 128],
                        rhs=xts(xv, half, kc2, nt),
                        start=(n == 1),
                        stop=(n == 3 * KC2),
                        perf_mode=DR,
                    )
            # evacuate 4 heads into two pair tiles; the upper psum half
            # partition-shifts down to the pair tile's base.
            for half_ps in range(2):
                nc.vector.tensor_scalar(
                    out=dst[2 * quad + half_ps][:, tt, qs : qs + 512],
                    in0=ps[half_ps * 64 : (half_ps + 1) * 64, :],
                    scalar1=b_sb[half_ps * 64 : (half_ps + 1) * 64, bl : bl + 1],
                    scalar2=s2,
                    op0=ADD,
                    op1=MULT,
                )

        # ---------- phase B building block ----------
        def b_head(h, j):
            """Attention for head h, q-tile j (512 queries)."""
            pr, hq = divmod(h, 2)
            r32 = hq * 32
            nk = 4 * j + 4  # causal: k-chunks 0..nk-1
            # au: 4 q-subchunks side by side, each [128 q, 64 attn + 1 denom].
            # The 4 causal accumulation groups share one PSUM bank, and a
            # start=True matmul marks the WHOLE 2KB bank pending-zero (which
            # would wipe sibling groups' partial sums) — so zero the tile with
            # one spanning matmul and accumulate everything with start=False.
            au = psum.tile([128, 4 * 65], F32, name=f"au_{h}_{j}", tag="au")
            nc.tensor.matmul(
                au[:],
                lhsT=z_sb[0:1, 0:128],
                rhs=z_sb[0:1, :],
                start=True,
                stop=True,
                skip_group_check=True,
            )

            def mk_st(pc):
                vp = max(0, 128 * (2 * pc) - 512 * j)
                st = psum.tile([128, 1024], F32, name=f"st_{h}_{j}_{pc}", tag="st")
                for u in range(2):
                    kc = 2 * pc + u
                    nc.tensor.matmul(
                        st[:, u * 512 + vp : (u + 1) * 512],
                        lhsT=kt8[pr][r32 : r32 + 32, :, kc * 128 : (kc + 1) * 128],
                        rhs=qt8[pr][r32 : r32 + 32, :, j * 512 + vp : (j + 1) * 512],
                        start=True,
                        stop=True,
                        perf_mode=DR,
                    )
                pt = pb.tile([128, 1024], FP16, name=f"pt_{h}_{j}_{pc}", tag="pt")
                st3 = st[:].rearrange("p (u c) -> p u c", u=2)
                pt3 = pt[:].rearrange("p (u c) -> p u c", u=2)
                nc.scalar.activation(
                    pt3[:, :, vp:512], st3[:, :, vp:512], EXP, scale=1.0 / 256.0
                )
                for u in range(2):
                    kc = 2 * pc + u
                    i = kc - 4 * j
                    if i >= 0:  # diagonal chunk: triangular 0/1 mask
                        c0 = u * 512 + 128 * i
                        nc.vector.tensor_tensor(
                            out=pt[:, c0 : c0 + 128],
                            in0=pt[:, c0 : c0 + 128],
                            in1=mask_sb[:, 0:128],
                            op=MULT,
                        )
                return pt

            def mk_av(pc, pt):
                for u in range(2):
                    kc = 2 * pc + u
                    for qs4 in range(4):
                        jq = 4 * j + qs4
                        if kc > jq:
                            continue
                        nc.tensor.matmul(
                            au[:, qs4 * 65 : qs4 * 65 + 65],
                            lhsT=pt[:, u * 512 + qs4 * 128 : u * 512 + (qs4 + 1) * 128],
                            rhs=vpr[kc][:, h * 65 : (h + 1) * 65],
                            start=False,
                            stop=(kc == jq),
                            skip_group_check=True,
                        )

            # software pipeline: issue st(pc+1) before AV(pc) so the PE keeps
            # running while ACT computes exp(pc).
            pts = {0: mk_st(0)}
            for pc in range(nk // 2):
                if pc + 1 < nk // 2:
                    pts[pc + 1] = mk_st(pc + 1)
                mk_av(pc, pts.pop(pc))

            # normalization: per-partition reciprocal of the 4 denominator
            # columns, then one fused multiply per q-subchunk.
            rcp = pbr.tile([128, 4], F32R, name=f"r_{h}_{j}", tag="r")
            with nc.allow_low_precision(
                reason="f32r output is bit-identical to f32 here"
            ):
                nc.vector.reciprocal(
                    out=rcp[:],
                    in_=au[:].rearrange("p (q c) -> p q c", c=65)[:, :, 64],
                )
            for qs4 in range(4):
                nc.vector.tensor_scalar(
                    out=at_nat[4 * j + qs4][:, h * 64 : (h + 1) * 64],
                    in0=au[:, qs4 * 65 : qs4 * 65 + 64],
                    scalar1=rcp[:, qs4 : qs4 + 1].bitcast(F32),
                    scalar2=None,
                    op0=MULT,
                )

        # ---------- transpose + phase C building blocks ----------
        def t_block(j, p):
            """Transpose at_nat[4j..4j+3] columns of pair p into at_sb[p]."""
            tp = psum.tile([128, 512], F32, name=f"tp_{j}_{p}", tag="pj")
            for qs4 in range(4):
                nc.tensor.matmul(
                    tp[:, qs4 * 128 : (qs4 + 1) * 128],
                    lhsT=at_nat[4 * j + qs4][:, p * 128 : (p + 1) * 128],
                    rhs=ident_sb[:],
                    start=True,
                    stop=True,
                )
            nc.vector.tensor_copy(
                out=at_sb[p][:, j * 512 : (j + 1) * 512], in_=tp[:]
            )

        def c_chunk(sc):
            osb = pc_pool.tile([128, DM], F32, name=f"osb_{sc}", tag="osb")
            for m in range(DM // 512):
                op_ps = psum.tile([128, 512], F32, name=f"ops_{sc}_{m}", tag="pj")
                for p in range(NPAIR):
                    nc.tensor.matmul(
                        op_ps[:],
                        lhsT=at_sb[p][:, sc * 128 : (sc + 1) * 128],
                        rhs=wo_sb[:, p * DM + m * 512 : p * DM + (m + 1) * 512],
                        start=(p == 0),
                        stop=(p == NPAIR - 1),
                    )
                nc.vector.tensor_copy(
                    out=osb[:, m * 512 : (m + 1) * 512], in_=op_ps[:]
                )
            nc.sync.dma_start(out=o[sc * 128 : (sc + 1) * 128, :], in_=osb[:])

        # ---------- interleaved schedule ----------
        def qk(half, bl, nt):
            return [
                lambda: a_qk_block("wq", qt8, bq_sb, 1.0 / 16.0, half, bl, nt),
                lambda: a_qk_block("wk", kt8, bk_sb, 0.5, half, bl, nt),
            ]

        # Minimal A prefix for B(h=0..3, j=0): Q/K pair tiles 0/1 (psum
        # blocks bl=0,1 of half 0) and V' chunks 0-3.
        for f in qk(0, 0, 0) + qk(0, 1, 0):
            f()
        for ss in range(4):
            a_v_chunk(0, ss)

        load_x(1)
        # remaining A work spread across B j=0..2 respecting column needs
        a_j0 = (
            qk(0, 2, 0) + qk(0, 3, 0)
            + [lambda ss=ss: a_v_chunk(0, ss) for ss in range(4, 8)]
            + qk(0, 0, 1) + qk(0, 1, 1)
        )
        a_j1 = (
            qk(0, 2, 1) + qk(0, 3, 1)
            + [lambda ss=ss: a_v_chunk(1, ss) for ss in range(4)]
            + qk(1, 0, 0) + qk(1, 1, 0) + qk(1, 2, 0) + qk(1, 3, 0)
        )
        a_j2 = (
            [lambda ss=ss: a_v_chunk(1, ss) for ss in range(4, 8)]
            + qk(1, 0, 1) + qk(1, 1, 1) + qk(1, 2, 1) + qk(1, 3, 1)
        )

        for h in range(8):
            b_head(h, 0)
            a_j0.pop(0)()
            if h % 2 == 1:
                a_j0.pop(0)()
        for p in range(NPAIR):
            t_block(0, p)
        nc.sync.dma_start(
            out=wo_sb[:].rearrange("p (pair c) -> p pair c", pair=NPAIR),
            in_=dram["wo"].rearrange("(pair p) c -> p pair c", p=128),
        )
        for h in range(8):
            b_head(h, 1)
            a_j1.pop(0)()
            a_j1.pop(0)()
        for p in range(NPAIR):
            t_block(1, p)
        for h in range(8):
            b_head(h, 2)
            a_j2.pop(0)()
            if h % 2 == 1:
                a_j2.pop(0)()
            if h % 2 == 1 and h // 2 < 4:
                c_chunk(h // 2)
        for p in range(NPAIR):
            t_block(2, p)
        for h in range(8):
            b_head(h, 3)
            c_chunk(4 + h)
        for p in range(NPAIR):
            t_block(3, p)
        for sc in range(12, 16):
            c_chunk(sc)

        if debug:
            nc.sync.dma_start(out=debug["d_qt8"], in_=qt8[0][:])
            nc.sync.dma_start(out=debug["d_kt8"], in_=kt8[0][:])
            nc.sync.dma_start(out=debug["d_vpr"], in_=vpr[0][:])
            nc.sync.dma_start(out=debug["d_an"], in_=at_nat[0][:])
            nc.sync.dma_start(out=debug["d_an5"], in_=at_nat[5][:])
            nc.sync.dma_start(out=debug["d_an15"], in_=at_nat[15][:])
            nc.sync.dma_start(out=debug["d_at"], in_=at_sb[0][:])


def _masks_np():
    # tri[r, c] = 1 where k_local <= q_local (unmasked on the diagonal block)
    r = np.arange(128)[:, None]
    c = np.arange(128)[None, :]
    return (c >= r).astype(np.float16)


def _qk_perm():
    """Column permutation mapping packed index bl*128 + (h%4)*32 + p to the
    natural column h*64 + t*32 + p (bl = (h//4)*2 + t)."""
    perm = np.empty(HV, np.int64)
    for h in range(8):
        for t in range(2):
            for p in range(32):
                bl = (h // 4) * 2 + t
                perm[bl * 128 + (h % 4) * 32 + p] = h * 64 + t * 32 + p
    return perm


def _split8(a):
    import ml_dtypes

    hi = np.asarray(a, np.float32).astype(ml_dtypes.float8_e4m3)
    lo = (np.asarray(a, np.float32) - hi.astype(np.float32)).astype(
        ml_dtypes.float8_e4m3
    )
    return hi, lo


def make_in_maps(input, Wq, bq, Wk, bk, Wv, Wo):
    scale = np.float32(1.0 / np.sqrt(D_K))
    masks = _masks_np()
    ident = np.eye(128, dtype=np.float16)
    perm = _qk_perm()
    input = np.asarray(input, np.float32)
    in_maps = []
    for c in range(NCORES):
        b, g = divmod(c, 2)
        cols = slice(g * HV, (g + 1) * HV)
        xh, xl = _split8(input[b].T)
        wqh, wql = _split8(np.asarray(Wq, np.float32)[:, cols][:, perm] * (scale * 256))
        wkh, wkl = _split8(np.asarray(Wk, np.float32)[:, cols][:, perm] * 32)
        wvh, wvl = _split8(np.asarray(Wv, np.float32)[:, cols] * 32)
        in_maps.append(
            {
                "xh": np.ascontiguousarray(xh),
                "xl": np.ascontiguousarray(xl),
                "wqh": np.ascontiguousarray(wqh),
                "wql": np.ascontiguousarray(wql),
                "wkh": np.ascontiguousarray(wkh),
                "wkl": np.ascontiguousarray(wkl),
                "wvh": np.ascontiguousarray(wvh),
                "wvl": np.ascontiguousarray(wvl),
                "bq": np.ascontiguousarray(
                    np.asarray(bq, np.float32)[cols][perm] * (scale * 256)
                ),
                "bk": np.ascontiguousarray(np.asarray(bk, np.float32)[cols][perm] * 32),
                "wo": np.ascontiguousarray(
                    np.asarray(Wo, np.float32)[g * HV : (g + 1) * HV, :]
                ).astype(np.float16),
                "masks": masks,
                "ident": ident,
            }
        )
    return in_maps


def _numpy_fallback(input, attn_mask, Wq, bq, Wk, bk, Wv, bv, Wo, bo):
    """Host fallback for non-causal masks (should not trigger in practice)."""
    x = np.asarray(input, np.float32)
    mask = np.asarray(attn_mask)
    B, S_, _ = x.shape
    scale = np.float32(1.0 / np.sqrt(D_K))
    out = np.empty((B, S_, D_MODEL), np.float32)
    for b in range(B):
        q = (x[b] @ Wq + bq).reshape(S_, N_HEAD, D_K)
        k = (x[b] @ Wk + bk).reshape(S_, N_HEAD, D_K)
        v = (x[b] @ Wv + bv).reshape(S_, N_HEAD, D_V)
        attn = np.empty((S_, N_HEAD, D_V), np.float32)
        for h in range(N_HEAD):
            score = (q[:, h] @ k[:, h].T) * scale
            score = np.where(mask, -np.inf, score)
            score -= score.max(axis=-1, keepdims=True)
            p = np.exp(score)
            p /= p.sum(axis=-1, keepdims=True)
            attn[:, h] = p @ v[:, h]
        out[b] = attn.reshape(S_, N_HEAD * D_V) @ Wo + bo
    return out


_CACHED_RUNNER = None


def _make_runner(nc):
    """Build the shard_map-jitted PJRT executor once; reuse across calls."""
    import jax
    from jax.sharding import Mesh, PartitionSpec
    from jax.experimental.shard_map import shard_map
    from concourse import bass2jax

    bass2jax.install_neuronx_cc_hook()
    partition_name = nc.partition_id_tensor.name if nc.partition_id_tensor else None
    in_names, out_names, out_avals, zero_outs = [], [], [], []
    for alloc in nc.m.functions[0].allocations:
        if not isinstance(alloc, mybir.MemoryLocationSet):
            continue
        name = alloc.memorylocations[0].name
        if alloc.kind == "ExternalInput":
            if name != partition_name:
                in_names.append(name)
        elif alloc.kind == "ExternalOutput":
            out_names.append(name)
            shape = tuple(alloc.tensor_shape)
            dtype = mybir.dt.np(alloc.dtype)
            out_avals.append(jax.core.ShapedArray(shape, dtype))
            zero_outs.append(np.zeros(shape, dtype))
    n_params = len(in_names)
    n_outs = len(out_avals)
    all_in_names = list(in_names) + list(out_names)
    if partition_name is not None:
        all_in_names.append(partition_name)

    def _body(*args):
        operands = list(args)
        if partition_name is not None:
            operands.append(bass2jax.partition_id_tensor())
        outs = bass2jax._bass_exec_p.bind(
            *operands,
            out_avals=tuple(out_avals),
            in_names=tuple(all_in_names),
            out_names=tuple(out_names),
            lowering_input_output_aliases=(),
            sim_require_finite=True,
            sim_require_nnan=True,
            nc=nc,
        )
        return tuple(outs)

    devices = jax.devices()[:NCORES]
    mesh = Mesh(np.asarray(devices), ("core",))
    sharded = jax.jit(
        shard_map(
            _body,
            mesh=mesh,
            in_specs=(PartitionSpec("core"),) * (n_params + n_outs),
            out_specs=(PartitionSpec("core"),) * n_outs,
            check_rep=False,
        ),
        donate_argnums=tuple(range(n_params, n_params + n_outs)),
        keep_unused=True,
    )

    def run(in_maps):
        concat_in = [
            np.concatenate(
                [np.asarray(in_maps[c][nm]) for c in range(NCORES)], axis=0
            )
            for nm in in_names
        ]
        concat_zeros = [
            np.zeros((NCORES * z.shape[0], *z.shape[1:]), z.dtype) for z in zero_outs
        ]
        out_arrs = sharded(*concat_in, *concat_zeros)
        return [
            {
                nm: np.asarray(out_arrs[i]).reshape(NCORES, *out_avals[i].shape)[c]
                for i, nm in enumerate(out_names)
            }
            for c in range(NCORES)
        ]

    return run


def kernel(input, attn_mask, Wq, bq, Wk, bk, Wv, bv, Wo, bo):
    causal = np.triu(np.ones((SEQ, SEQ), bool), k=1)
    if not np.array_equal(np.asarray(attn_mask), causal):
        return _numpy_fallback(input, attn_mask, Wq, bq, Wk, bk, Wv, bv, Wo, bo)

    global _CACHED_NC, _CACHED_RUNNER
    if _CACHED_NC is None:
        _CACHED_NC = _build_nc()

    in_maps = make_in_maps(input, Wq, bq, Wk, bk, Wv, Wo)
    try:
        if _CACHED_RUNNER is None:
            _CACHED_RUNNER = _make_runner(_CACHED_NC)
        outs = _CACHED_RUNNER(in_maps)
    except Exception:
        # jit-caching fast path failed (e.g. jax version skew) — use the
        # stock executor.
        _CACHED_RUNNER = None
        outs = bass_utils.run_bass_kernel_spmd(
            _CACHED_NC, in_maps, core_ids=list(range(NCORES))
        ).results

    corr = (
        np.asarray(bv, np.float32) @ np.asarray(Wo, np.float32)
        + np.asarray(bo, np.float32)
    ).astype(np.float32)
    out = np.empty((BATCH, SEQ, D_MODEL), np.float32)
    for b in range(BATCH):
        out[b] = outs[2 * b]["o"] + outs[2 * b + 1]["o"] + corr[None, :]
    return out


# revision 43
# speedup vs baseline: 1.1242x; 1.0233x over previous
"""Multi-head causal attention (B=4, S=2048, D=1024, H=16, dk=dv=64) on 8 NeuronCores.

Sharding: core c -> (batch b = c//2, head-group g = c%2 of 8 heads).
Each core computes Q/K/V projections for its batch restricted to its 8 heads,
causal softmax attention, and a partial output projection with its 512 rows of
Wo.  The host sums the two partials per batch and adds the constant correction
bv @ Wo + bo (bv passes through attention linearly because softmax rows sum
to 1).

v4 highlights (per core):
  - Projections run as compensated-fp8 DoubleRow matmuls: host splits x^T and
    the (range-scaled) weights into fp8 hi+lo pairs; x@W ~ xh@Wh + xh@Wl +
    xl@Wh costs 3 DoubleRow passes = 0.75x the f32r cost (measured end-to-end
    error 0.1%).
  - Q^T/K^T are emitted directly in the DoubleRow-packed fp8 layout
    ([128 = 4 heads x 32 dk, 2 dk-halves, S]) by permuting W's columns on the
    host, so the score matmuls run fp8 DoubleRow at 0.5 cycles/row with no
    repacking.  Scale 16 per side, undone by the exp's 1/256 input scale.
  - exp on ACT (f32 PSUM scores -> fp16 P^T); causal-masked diagonal chunks
    multiplied by a triangular 0/1 mask on the DVE (fp16 4x mode).
  - AV runs in natural orientation (out [128 q, 65]) using all 128 PE
    partitions (2x fewer cycles than transposed) with a ones column for the
    softmax denominator; normalization is a per-partition reciprocal + one
    fused DVE tensor_scalar per head.
  - attn tiles are transposed for the output projection via identity-matmul
    on the PE (128 cycles each).
  - phases are emitted interleaved so ACT exp work overlaps the PE-heavy
    projections and output projection; score tiles are software-pipelined.
"""

import numpy as np
from contextlib import ExitStack

import concourse.bass as bass
import concourse.mybir as mybir
import concourse.tile as tile
from concourse import bacc, bass_utils

N_HEAD, D_MODEL, D_K, D_V = 16, 1024, 64, 64
BATCH, SEQ = 4, 2048
NCORES = 8
S = SEQ
DM = D_MODEL
HV = 8 * D_V          # 512 local head-value columns per core
KC2 = DM // 256       # 4 DoubleRow contraction chunks
NPAIR = 4             # local head pairs
NQT = S // 512        # 4 q-tiles
F32 = mybir.dt.float32
F32R = mybir.dt.float32r
FP16 = mybir.dt.float16
FP8 = mybir.dt.float8e4
DR = mybir.MatmulPerfMode.DoubleRow

_CACHED_NC = None


def _build_nc(nbody=1):
    nc = bacc.Bacc("TRN2", target_bir_lowering=False, debug=False)

    dram = {}
    for nm in ("xh", "xl"):
        dram[nm] = nc.dram_tensor(nm, [DM, S], FP8, kind="ExternalInput").ap()
    for nm in ("wqh", "wql", "wkh", "wkl", "wvh", "wvl"):
        dram[nm] = nc.dram_tensor(nm, [DM, HV], FP8, kind="ExternalInput").ap()
    dram["wo"] = nc.dram_tensor("wo", [HV, DM], FP16, kind="ExternalInput").ap()
    dram["bq"] = nc.dram_tensor("bq", [HV], F32, kind="ExternalInput").ap()
    dram["bk"] = nc.dram_tensor("bk", [HV], F32, kind="ExternalInput").ap()
    dram["masks"] = nc.dram_tensor("masks", [128, 128], FP16, kind="ExternalInput").ap()
    dram["ident"] = nc.dram_tensor("ident", [128, 128], FP16, kind="ExternalInput").ap()
    o = nc.dram_tensor("o", [S, DM], F32, kind="ExternalOutput").ap()

    with tile.TileContext(nc) as tc:
        for _ in range(nbody):
            _build_kernel(tc, nc, dram, o, debug=nbody == -1)
    nc.compile()
    return nc


def _build_debug_nc():
    nc = bacc.Bacc("TRN2", target_bir_lowering=False, debug=False)
    dram = {}
    for nm in ("xh", "xl"):
        dram[nm] = nc.dram_tensor(nm, [DM, S], FP8, kind="ExternalInput").ap()
    for nm in ("wqh", "wql", "wkh", "wkl", "wvh", "wvl"):
        dram[nm] = nc.dram_tensor(nm, [DM, HV], FP8, kind="ExternalInput").ap()
    dram["wo"] = nc.dram_tensor("wo", [HV, DM], FP16, kind="ExternalInput").ap()
    dram["bq"] = nc.dram_tensor("bq", [HV], F32, kind="ExternalInput").ap()
    dram["bk"] = nc.dram_tensor("bk", [HV], F32, kind="ExternalInput").ap()
    dram["masks"] = nc.dram_tensor("masks", [128, 128], FP16, kind="ExternalInput").ap()
    dram["ident"] = nc.dram_tensor("ident", [128, 128], FP16, kind="ExternalInput").ap()
    o = nc.dram_tensor("o", [S, DM], F32, kind="ExternalOutput").ap()
    dbg = {
        "d_qt8": nc.dram_tensor("d_qt8", [64, 2, S], FP8, kind="ExternalOutput").ap(),
        "d_kt8": nc.dram_tensor("d_kt8", [64, 2, S], FP8, kind="ExternalOutput").ap(),
        "d_vpr": nc.dram_tensor("d_vpr", [128, 8 * 65], FP16, kind="ExternalOutput").ap(),
        "d_an": nc.dram_tensor("d_an", [128, HV], FP16, kind="ExternalOutput").ap(),
        "d_an5": nc.dram_tensor("d_an5", [128, HV], FP16, kind="ExternalOutput").ap(),
        "d_an15": nc.dram_tensor("d_an15", [128, HV], FP16, kind="ExternalOutput").ap(),
        "d_at": nc.dram_tensor("d_at", [128, S], FP16, kind="ExternalOutput").ap(),
    }
    with tile.TileContext(nc) as tc:
        _build_kernel(tc, nc, dram, o, debug=dbg)
    nc.compile()
    return nc


def _build_kernel(tc, nc, dram, o, debug=None):
    EXP = mybir.ActivationFunctionType.Exp
    MULT = mybir.AluOpType.mult
    ADD = mybir.AluOpType.add

    with ExitStack() as ctx:
        # ---- persistent tensors (live across phases) ----
        pp = ctx.enter_context(tc.tile_pool(name="persist", bufs=1))
        # packed fp8 Q^T/K^T: quad tile q holds heads 4q..4q+3 at base
        # partition (h%4)*32; free dims = (dk-half t, s).  Score matmuls pass
        # an explicit tile_position so base partition 96 is usable.
        qt8 = [pp.tile([128, 2, S], FP8, name=f"qt8_{q}", tag=f"qt8_{q}") for q in range(2)]
        kt8 = [pp.tile([128, 2, S], FP8, name=f"kt8_{q}", tag=f"kt8_{q}") for q in range(2)]
        vpr = [
            pp.tile([128, 8 * 65], FP16, name=f"vp{sc}", tag=f"vp{sc}")
            for sc in range(S // 128)
        ]
        at_nat = [
            pp.tile([128, HV], FP16, name=f"an{sc}", tag=f"an{sc}")
            for sc in range(S // 128)
        ]
        at_sb = [
            pp.tile([128, S], FP16, name=f"at{p}", tag=f"at{p}")
            for p in range(NPAIR)
        ]
        wo_sb = pp.tile([128, NPAIR * DM], FP16, name="wo_sb", tag="wo_sb")
        mask_sb = pp.tile([128, 128], FP16, name="mask_sb", tag="mask_sb")
        ident_sb = pp.tile([128, 128], FP16, name="ident_sb", tag="ident_sb")
        bq_sb = pp.tile([128, NPAIR], F32, name="bq_sb", tag="bq_sb")
        bk_sb = pp.tile([128, NPAIR], F32, name="bk_sb", tag="bk_sb")
        # zero row for the au-zeroing matmul (see b_head)
        z_sb = pp.tile([1, 4 * 65], FP16, name="z_sb", tag="z_sb")
        # e^(1/256) base for Pool-engine exp offload (pow(base, st) = exp(st/256))
        e256_sb = pp.tile([128, 1024], F32, name="e256_sb", tag="e256_sb")
        # weights: per tensor a [128, KC2 * 2 * 512] fp8 tile, chunk kc2 at
        # [:, kc2, t, :]
        pa = ctx.enter_context(tc.tile_pool(name="pa", bufs=1))
        w_sb = {
            nm: pa.tile([128, KC2, 2, HV], FP8, name=f"{nm}_sb", tag=f"{nm}_sb")
            for nm in ("wqh", "wql", "wkh", "wkl", "wvh", "wvl")
        }
        pax = ctx.enter_context(tc.tile_pool(name="pa_x", bufs=8))
        pb = ctx.enter_context(tc.tile_pool(name="pb", bufs=8))
        pbr = ctx.enter_context(tc.tile_pool(name="pb_r", bufs=4))
        pc_pool = ctx.enter_context(tc.tile_pool(name="pc", bufs=3))
        # PSUM: pj 2x1 + st 2x2 + au 2x1 = 8 banks.
        psum = ctx.enter_context(tc.tile_pool(name="psum", bufs=2, space="PSUM"))

        nc.gpsimd.memset(z_sb[:], 0.0)
        nc.gpsimd.memset(e256_sb[:], float(np.exp(1.0 / 256.0)))
        for sc in range(S // 128):
            nc.gpsimd.memset(vpr[sc][:], 1.0)

        SH = S // 2

        # ---------- DMA (few, large transfers: HWDGE dispatch is ~0.6us per
        # DMA, so consolidate each tensor-half into one descriptor) ----------
        xtile = {}

        def load_x_q(half, nt):
            """load one 512-column slice of x (hi+lo)."""
            s0 = half * SH + nt * 512
            for v in ("xh", "xl"):
                t = pax.tile([128, KC2, 2, 512], FP8, name=f"xt_{v}_{half}_{nt}", tag="xt")
                nc.sync.dma_start(
                    out=t[:],
                    in_=dram[v][:, s0 : s0 + 512].rearrange(
                        "(k t p) s -> p k t s", p=128, t=2
                    ),
                )
                xtile[(v, half, nt)] = t

        def load_x(half):
            for nt in range(2):
                load_x_q(half, nt)

        def xts(v, half, kc2, nt):
            return xtile[(v, half, nt)][:, kc2]

        def load_w(nm):
            nc.sync.dma_start(
                out=w_sb[nm][:],
                in_=dram[nm].rearrange("(k t p) c -> p k t c", p=128, t=2),
            )

        load_x_q(0, 0)
        for nm in ("wqh", "wql", "wkh", "wkl"):
            load_w(nm)
        nc.sync.dma_start(
            out=bq_sb[:], in_=dram["bq"].rearrange("(bl r) -> r bl", r=128)
        )
        nc.sync.dma_start(
            out=bk_sb[:], in_=dram["bk"].rearrange("(bl r) -> r bl", r=128)
        )
        load_w("wvh")
        load_w("wvl")
        load_x_q(0, 1)
        nc.sync.dma_start(out=mask_sb[:], in_=dram["masks"])
        nc.sync.dma_start(out=ident_sb[:], in_=dram["ident"])

        # compensated-fp8 product passes: (xh,Wh), (xh,Wl), (xl,Wh)
        COMB = (("xh", "h"), ("xh", "l"), ("xl", "h"))

        # ---------- phase A building blocks ----------
        def a_v_chunk(half, ss):
            """V' tile for s-chunk (half*8 + ss): out [128 s, 512 cols]."""
            sc = half * (SH // 128) + ss
            vp_ps = psum.tile([128, 512], F32, name=f"vps_{sc}", tag="pj")
            n = 0
            for kc2 in range(KC2):
                for xv, wv_ in COMB:
                    n += 1
                    nc.tensor.matmul(
                        vp_ps[:],
                        lhsT=xts(xv, half, kc2, ss // 4)[:, :, (ss % 4) * 128 : (ss % 4 + 1) * 128],
                        rhs=w_sb["wv" + wv_][:, kc2],
                        start=(n == 1),
                        stop=(n == 3 * KC2),
                        perf_mode=DR,
                    )
            if False:
                nc.scalar.activation(
                    vpr[sc][:].rearrange("p (h c) -> p h c", h=8)[:, :, 0:64],
                    vp_ps[:].rearrange("p (h c) -> p h c", h=8),
                    mybir.ActivationFunctionType.Copy,
                    scale=1.0 / 32.0,
                )
            else:
                nc.vector.tensor_scalar(
                    out=vpr[sc][:].rearrange("p (h c) -> p h c", h=8)[:, :, 0:64],
                    in0=vp_ps[:].rearrange("p (h c) -> p h c", h=8),
                    scalar1=1.0 / 32.0,
                    scalar2=None,
                    op0=MULT,
                )

        def a_qk_block(wch, dst, b_sb, s2, half, bl, nt):
            """One packed-fp8 projection block: psum [128 cols', 512 s] ->
            fp8 quad tile. bl = quad*2 + t."""
            quad, tt = divmod(bl, 2)
            s0 = half * SH
            qs = s0 + nt * 512
            ps = psum.tile([128, 512], F32, name=f"qk_{wch}_{bl}_{qs}", tag="pj")
            n = 0
            for kc2 in range(KC2):
                for xv, wv_ in COMB:
                    n += 1
                    nc.tensor.matmul(
                        ps[:],
                        lhsT=w_sb[wch + wv_][:, kc2, :, bl * 128 : (bl + 1) * 128],
                        rhs=xts(xv, half, kc2, nt),
                        start=(n == 1),
                        stop=(n == 3 * KC2),
                        perf_mode=DR,
                    )
            nc.vector.tensor_scalar(
                out=dst[quad][:, tt, qs : qs + 512],
                in0=ps[:],
                scalar1=b_sb[:, bl : bl + 1],
                scalar2=s2,
                op0=ADD,
                op1=MULT,
            )

        # ---------- phase B building block ----------
        def b_head(h, j):
            """Attention for head h, q-tile j (512 queries)."""
            quad, hq = divmod(h, 4)
            r32 = hq * 32
            nk = 4 * j + 4  # causal: k-chunks 0..nk-1
            # au: 4 q-subchunks side by side, each [128 q, 64 attn + 1 denom].
            # The 4 causal accumulation groups share one PSUM bank, and a
            # start=True matmul marks the WHOLE 2KB bank pending-zero (which
            # would wipe sibling groups' partial sums) — so zero the tile with
            # one spanning matmul and accumulate everything with start=False.
            au = psum.tile([128, 4 * 65], F32, name=f"au_{h}_{j}", tag="au")
            nc.tensor.matmul(
                au[:],
                lhsT=z_sb[0:1, 0:128],
                rhs=z_sb[0:1, :],
                start=True,
                stop=True,
                skip_group_check=True,
            )

            def mk_st(pc):
                vp = max(0, 128 * (2 * pc) - 512 * j)
                st = psum.tile([128, 1024], F32, name=f"st_{h}_{j}_{pc}", tag="st")
                for u in range(2):
                    kc = 2 * pc + u
                    nc.tensor.matmul(
                        st[:, u * 512 + vp : (u + 1) * 512],
                        lhsT=kt8[quad][r32 : r32 + 32, :, kc * 128 : (kc + 1) * 128],
                        rhs=qt8[quad][r32 : r32 + 32, :, j * 512 + vp : (j + 1) * 512],
                        start=True,
                        stop=True,
                        perf_mode=DR,
                        tile_position=(r32, 0),
                    )
                pt = pb.tile([128, 1024], FP16, name=f"pt_{h}_{j}_{pc}", tag="pt")
                st3 = st[:].rearrange("p (u c) -> p u c", u=2)
                pt3 = pt[:].rearrange("p (u c) -> p u c", u=2)
                nc.scalar.activation(
                    pt3[:, :, vp:512], st3[:, :, vp:512], EXP, scale=1.0 / 256.0
                )
                for u in range(2):
                    kc = 2 * pc + u
                    i = kc - 4 * j
                    if i >= 0:  # diagonal chunk: triangular 0/1 mask
                        c0 = u * 512 + 128 * i
                        nc.vector.tensor_tensor(
                            out=pt[:, c0 : c0 + 128],
                            in0=pt[:, c0 : c0 + 128],
                            in1=mask_sb[:, 0:128],
                            op=MULT,
                        )
                return pt

            def mk_av(pc, pt):
                for u in range(2):
                    kc = 2 * pc + u
                    for qs4 in range(4):
                        jq = 4 * j + qs4
                        if kc > jq:
                            continue
                        nc.tensor.matmul(
                            au[:, qs4 * 65 : qs4 * 65 + 65],
                            lhsT=pt[:, u * 512 + qs4 * 128 : u * 512 + (qs4 + 1) * 128],
                            rhs=vpr[kc][:, h * 65 : (h + 1) * 65],
                            start=False,
                            stop=(kc == jq),
                            skip_group_check=True,
                        )

            # software pipeline: issue st(pc+1) before AV(pc) so the PE keeps
            # running while ACT computes exp(pc).
            pts = {0: mk_st(0)}
            for pc in range(nk // 2):
                if pc + 1 < nk // 2:
                    pts[pc + 1] = mk_st(pc + 1)
                mk_av(pc, pts.pop(pc))

            # normalization: per-partition reciprocal of the 4 denominator
            # columns, then one fused multiply per q-subchunk.
            rcp = pbr.tile([128, 4], F32R, name=f"r_{h}_{j}", tag="r")
            with nc.allow_low_precision(
                reason="f32r output is bit-identical to f32 here"
            ):
                nc.vector.reciprocal(
                    out=rcp[:],
                    in_=au[:].rearrange("p (q c) -> p q c", c=65)[:, :, 64],
                )
            for qs4 in range(4):
                if False:
                    nc.scalar.activation(
                        at_nat[4 * j + qs4][:, h * 64 : (h + 1) * 64],
                        au[:, qs4 * 65 : qs4 * 65 + 64],
                        mybir.ActivationFunctionType.Copy,
                        scale=rcp[:, qs4 : qs4 + 1].bitcast(F32),
                    )
                else:
                    nc.vector.tensor_scalar(
                        out=at_nat[4 * j + qs4][:, h * 64 : (h + 1) * 64],
                        in0=au[:, qs4 * 65 : qs4 * 65 + 64],
                        scalar1=rcp[:, qs4 : qs4 + 1].bitcast(F32),
                        scalar2=None,
                        op0=MULT,
                    )

        # ---------- transpose + phase C building blocks ----------
        def t_block(j, p):
            """Transpose at_nat[4j..4j+3] columns of pair p into at_sb[p]."""
            tp = psum.tile([128, 512], F32, name=f"tp_{j}_{p}", tag="pj")
            for qs4 in range(4):
                nc.tensor.matmul(
                    tp[:, qs4 * 128 : (qs4 + 1) * 128],
                    lhsT=at_nat[4 * j + qs4][:, p * 128 : (p + 1) * 128],
                    rhs=ident_sb[:],
                    start=True,
                    stop=True,
                )
            nc.vector.tensor_copy(
                out=at_sb[p][:, j * 512 : (j + 1) * 512], in_=tp[:]
            )

        def c_chunk(sc):
            osb = pc_pool.tile([128, DM], F32, name=f"osb_{sc}", tag="osb")
            for m in range(DM // 512):
                op_ps = psum.tile([128, 512], F32, name=f"ops_{sc}_{m}", tag="pj")
                for p in range(NPAIR):
                    nc.tensor.matmul(
                        op_ps[:],
                        lhsT=at_sb[p][:, sc * 128 : (sc + 1) * 128],
                        rhs=wo_sb[:, p * DM + m * 512 : p * DM + (m + 1) * 512],
                        start=(p == 0),
                        stop=(p == NPAIR - 1),
                    )
                nc.vector.tensor_copy(
                    out=osb[:, m * 512 : (m + 1) * 512], in_=op_ps[:]
                )
            nc.sync.dma_start(out=o[sc * 128 : (sc + 1) * 128, :], in_=osb[:])

        # ---------- interleaved schedule ----------
        def qk(half, bl, nt):
            return [
                lambda: a_qk_block("wq", qt8, bq_sb, 1.0 / 16.0, half, bl, nt),
                lambda: a_qk_block("wk", kt8, bk_sb, 0.5, half, bl, nt),
            ]

        # Minimal A prefix for B(h=0..3, j=0): Q/K pair tiles 0/1 (psum
        # blocks bl=0,1 of half 0) and V' chunks 0-3.
        for f in qk(0, 0, 0) + qk(0, 1, 0):
            f()
        for ss in range(4):
            a_v_chunk(0, ss)

        load_x(1)
        # remaining A work spread across B j=0..2 respecting column needs
        a_j0 = (
            qk(0, 2, 0) + qk(0, 3, 0)
            + [lambda ss=ss: a_v_chunk(0, ss) for ss in range(4, 8)]
            + qk(0, 0, 1) + qk(0, 1, 1)
        )
        a_j1 = (
            qk(0, 2, 1) + qk(0, 3, 1)
            + [lambda ss=ss: a_v_chunk(1, ss) for ss in range(4)]
            + qk(1, 0, 0) + qk(1, 1, 0) + qk(1, 2, 0) + qk(1, 3, 0)
        )
        a_j2 = (
            [lambda ss=ss: a_v_chunk(1, ss) for ss in range(4, 8)]
            + qk(1, 0, 1) + qk(1, 1, 1) + qk(1, 2, 1) + qk(1, 3, 1)
        )

        for h in range(8):
            b_head(h, 0)
            a_j0.pop(0)()
            if h % 2 == 1:
                a_j0.pop(0)()
        for p in range(NPAIR):
            t_block(0, p)
        nc.sync.dma_start(
            out=wo_sb[:].rearrange("p (pair c) -> p pair c", pair=NPAIR),
            in_=dram["wo"].rearrange("(pair p) c -> p pair c", p=128),
        )
        for h in range(8):
            b_head(h, 1)
            a_j1.pop(0)()
            a_j1.pop(0)()
        for p in range(NPAIR):
            t_block(1, p)
        for h in range(8):
            b_head(h, 2)
            a_j2.pop(0)()
            if h % 2 == 1:
                a_j2.pop(0)()
            if h % 2 == 1 and h // 2 < 4:
                c_chunk(h // 2)
        for p in range(NPAIR):
            t_block(2, p)
        for h in range(8):
            b_head(h, 3)
            c_chunk(4 + h)
        for p in range(NPAIR):
            t_block(3, p)
        for sc in range(12, 16):
            c_chunk(sc)

        if debug:
            nc.sync.dma_start(out=debug["d_qt8"], in_=qt8[0][:])
            nc.sync.dma_start(out=debug["d_kt8"], in_=kt8[0][:])
            nc.sync.dma_start(out=debug["d_vpr"], in_=vpr[0][:])
            nc.sync.dma_start(out=debug["d_an"], in_=at_nat[0][:])
            nc.sync.dma_start(out=debug["d_an5"], in_=at_nat[5][:])
            nc.sync.dma_start(out=debug["d_an15"], in_=at_nat[15][:])
            nc.sync.dma_start(out=debug["d_at"], in_=at_sb[0][:])


def _masks_np():
    # tri[r, c] = 1 where k_local <= q_local (unmasked on the diagonal block)
    r = np.arange(128)[:, None]
    c = np.arange(128)[None, :]
    return (c >= r).astype(np.float16)


def _qk_perm():
    """Column permutation mapping packed index bl*128 + (h%4)*32 + p to the
    natural column h*64 + t*32 + p (bl = (h//4)*2 + t)."""
    perm = np.empty(HV, np.int64)
    for h in range(8):
        for t in range(2):
            for p in range(32):
                bl = (h // 4) * 2 + t
                perm[bl * 128 + (h % 4) * 32 + p] = h * 64 + t * 32 + p
    return perm


def _split8(a):
    import ml_dtypes

    hi = np.asarray(a, np.float32).astype(ml_dtypes.float8_e4m3)
    lo = (np.asarray(a, np.float32) - hi.astype(np.float32)).astype(
        ml_dtypes.float8_e4m3
    )
    return hi, lo


def make_in_maps(input, Wq, bq, Wk, bk, Wv, Wo):
    scale = np.float32(1.0 / np.sqrt(D_K))
    masks = _masks_np()
    ident = np.eye(128, dtype=np.float16)
    perm = _qk_perm()
    input = np.asarray(input, np.float32)
    in_maps = []
    for c in range(NCORES):
        b, g = divmod(c, 2)
        cols = slice(g * HV, (g + 1) * HV)
        xh, xl = _split8(input[b].T)
        wqh, wql = _split8(np.asarray(Wq, np.float32)[:, cols][:, perm] * (scale * 256))
        wkh, wkl = _split8(np.asarray(Wk, np.float32)[:, cols][:, perm] * 32)
        wvh, wvl = _split8(np.asarray(Wv, np.float32)[:, cols] * 32)
        in_maps.append(
            {
                "xh": np.ascontiguousarray(xh),
                "xl": np.ascontiguousarray(xl),
                "wqh": np.ascontiguousarray(wqh),
                "wql": np.ascontiguousarray(wql),
                "wkh": np.ascontiguousarray(wkh),
                "wkl": np.ascontiguousarray(wkl),
                "wvh": np.ascontiguousarray(wvh),
                "wvl": np.ascontiguousarray(wvl),
                "bq": np.ascontiguousarray(
                    np.asarray(bq, np.float32)[cols][perm] * (scale * 256)
                ),
                "bk": np.ascontiguousarray(np.asarray(bk, np.float32)[cols][perm] * 32),
                "wo": np.ascontiguousarray(
                    np.asarray(Wo, np.float32)[g * HV : (g + 1) * HV, :]
                ).astype(np.float16),
                "masks": masks,
                "ident": ident,
            }
        )
    return in_maps


def _numpy_fallback(input, attn_mask, Wq, bq, Wk, bk, Wv, bv, Wo, bo):
    """Host fallback for non-causal masks (should not trigger in practice)."""
    x = np.asarray(input, np.float32)
    mask = np.asarray(attn_mask)
    B, S_, _ = x.shape
    scale = np.float32(1.0 / np.sqrt(D_K))
    out = np.empty((B, S_, D_MODEL), np.float32)
    for b in range(B):
        q = (x[b] @ Wq + bq).reshape(S_, N_HEAD, D_K)
        k = (x[b] @ Wk + bk).reshape(S_, N_HEAD, D_K)
        v = (x[b] @ Wv + bv).reshape(S_, N_HEAD, D_V)
        attn = np.empty((S_, N_HEAD, D_V), np.float32)
        for h in range(N_HEAD):
            score = (q[:, h] @ k[:, h].T) * scale
            score = np.where(mask, -np.inf, score)
            score -= score.max(axis=-1, keepdims=True)
            p = np.exp(score)
            p /= p.sum(axis=-1, keepdims=True)
            attn[:, h] = p @ v[:, h]
        out[b] = attn.reshape(S_, N_HEAD * D_V) @ Wo + bo
    return out


_CACHED_RUNNER = None


def _make_runner(nc):
    """Build the shard_map-jitted PJRT executor once; reuse across calls."""
    import jax
    from jax.sharding import Mesh, PartitionSpec
    from jax.experimental.shard_map import shard_map
    from concourse import bass2jax

    bass2jax.install_neuronx_cc_hook()
    partition_name = nc.partition_id_tensor.name if nc.partition_id_tensor else None
    in_names, out_names, out_avals, zero_outs = [], [], [], []
    for alloc in nc.m.functions[0].allocations:
        if not isinstance(alloc, mybir.MemoryLocationSet):
            continue
        name = alloc.memorylocations[0].name
        if alloc.kind == "ExternalInput":
            if name != partition_name:
                in_names.append(name)
        elif alloc.kind == "ExternalOutput":
            out_names.append(name)
            shape = tuple(alloc.tensor_shape)
            dtype = mybir.dt.np(alloc.dtype)
            out_avals.append(jax.core.ShapedArray(shape, dtype))
            zero_outs.append(np.zeros(shape, dtype))
    n_params = len(in_names)
    n_outs = len(out_avals)
    all_in_names = list(in_names) + list(out_names)
    if partition_name is not None:
        all_in_names.append(partition_name)

    def _body(*args):
        operands = list(args)
        if partition_name is not None:
            operands.append(bass2jax.partition_id_tensor())
        outs = bass2jax._bass_exec_p.bind(
            *operands,
            out_avals=tuple(out_avals),
            in_names=tuple(all_in_names),
            out_names=tuple(out_names),
            lowering_input_output_aliases=(),
            sim_require_finite=True,
            sim_require_nnan=True,
            nc=nc,
        )
        return tuple(outs)

    devices = jax.devices()[:NCORES]
    mesh = Mesh(np.asarray(devices), ("core",))
    sharded = jax.jit(
        shard_map(
            _body,
            mesh=mesh,
            in_specs=(PartitionSpec("core"),) * (n_params + n_outs),
            out_specs=(PartitionSpec("core"),) * n_outs,
            check_rep=False,
        ),
        donate_argnums=tuple(range(n_params, n_params + n_outs)),
        keep_unused=True,
    )

    def run(in_maps):
        concat_in = [
            np.concatenate(
                [np.asarray(in_maps[c][nm]) for c in range(NCORES)], axis=0
            )
            for nm in in_names
        ]
        concat_zeros = [
            np.zeros((NCORES * z.shape[0], *z.shape[1:]), z.dtype) for z in zero_outs
        ]
        out_arrs = sharded(*concat_in, *concat_zeros)
        return [
            {
                nm: np.asarray(out_arrs[i]).reshape(NCORES, *out_avals[i].shape)[c]
                for i, nm in enumerate(out_names)
            }
            for c in range(NCORES)
        ]

    return run


def kernel(input, attn_mask, Wq, bq, Wk, bk, Wv, bv, Wo, bo):
    causal = np.triu(np.ones((SEQ, SEQ), bool), k=1)
    if not np.array_equal(np.asarray(attn_mask), causal):
        return _numpy_fallback(input, attn_mask, Wq, bq, Wk, bk, Wv, bv, Wo, bo)

    global _CACHED_NC, _CACHED_RUNNER
    if _CACHED_NC is None:
        _CACHED_NC = _build_nc()

    in_maps = make_in_maps(input, Wq, bq, Wk, bk, Wv, Wo)
    try:
        if _CACHED_RUNNER is None:
            _CACHED_RUNNER = _make_runner(_CACHED_NC)
        outs = _CACHED_RUNNER(in_maps)
    except Exception:
        # jit-caching fast path failed (e.g. jax version skew) — use the
        # stock executor.
        _CACHED_RUNNER = None
        outs = bass_utils.run_bass_kernel_spmd(
            _CACHED_NC, in_maps, core_ids=list(range(NCORES))
        ).results

    corr = (
        np.asarray(bv, np.float32) @ np.asarray(Wo, np.float32)
        + np.asarray(bo, np.float32)
    ).astype(np.float32)
    out = np.empty((BATCH, SEQ, D_MODEL), np.float32)
    for b in range(BATCH):
        out[b] = outs[2 * b]["o"] + outs[2 * b + 1]["o"] + corr[None, :]
    return out


# revision 46
# speedup vs baseline: 1.1242x; 1.0000x over previous
"""Multi-head causal attention (B=4, S=2048, D=1024, H=16, dk=dv=64) on 8 NeuronCores.

Sharding: core c -> (batch b = c//2, head-group g = c%2 of 8 heads).
Each core computes Q/K/V projections for its batch restricted to its 8 heads,
causal softmax attention, and a partial output projection with its 512 rows of
Wo.  The host sums the two partials per batch and adds the constant correction
bv @ Wo + bo (bv passes through attention linearly because softmax rows sum
to 1).

v4 highlights (per core):
  - Projections run as compensated-fp8 DoubleRow matmuls: host splits x^T and
    the (range-scaled) weights into fp8 hi+lo pairs; x@W ~ xh@Wh + xh@Wl +
    xl@Wh costs 3 DoubleRow passes = 0.75x the f32r cost (measured end-to-end
    error 0.1%).
  - Q^T/K^T are emitted directly in the DoubleRow-packed fp8 layout
    ([128 = 4 heads x 32 dk, 2 dk-halves, S]) by permuting W's columns on the
    host, so the score matmuls run fp8 DoubleRow at 0.5 cycles/row with no
    repacking.  Scale 16 per side, undone by the exp's 1/256 input scale.
  - exp on ACT (f32 PSUM scores -> fp16 P^T); causal-masked diagonal chunks
    multiplied by a triangular 0/1 mask on the DVE (fp16 4x mode).
  - AV runs in natural orientation (out [128 q, 65]) using all 128 PE
    partitions (2x fewer cycles than transposed) with a ones column for the
    softmax denominator; normalization is a per-partition reciprocal + one
    fused DVE tensor_scalar per head.
  - attn tiles are transposed for the output projection via identity-matmul
    on the PE (128 cycles each).
  - phases are emitted interleaved so ACT exp work overlaps the PE-heavy
    projections and output projection; score tiles are software-pipelined.
"""

import numpy as np
from contextlib import ExitStack

import concourse.bass as bass
import concourse.mybir as mybir
import concourse.tile as tile
from concourse import bacc, bass_utils

N_HEAD, D_MODEL, D_K, D_V = 16, 1024, 64, 64
BATCH, SEQ = 4, 2048
NCORES = 8
S = SEQ
DM = D_MODEL
HV = 8 * D_V          # 512 local head-value columns per core
KC2 = DM // 256       # 4 DoubleRow contraction chunks
NPAIR = 4             # local head pairs
NQT = S // 512        # 4 q-tiles
F32 = mybir.dt.float32
F32R = mybir.dt.float32r
FP16 = mybir.dt.float16
FP8 = mybir.dt.float8e4
DR = mybir.MatmulPerfMode.DoubleRow

_CACHED_NC = None


def _build_nc(nbody=1):
    nc = bacc.Bacc("TRN2", target_bir_lowering=False, debug=False)

    dram = {}
    for nm in ("xh", "xl"):
        dram[nm] = nc.dram_tensor(nm, [DM, S], FP8, kind="ExternalInput").ap()
    for nm in ("wqh", "wql", "wkh", "wkl", "wvh", "wvl"):
        dram[nm] = nc.dram_tensor(nm, [DM, HV], FP8, kind="ExternalInput").ap()
    dram["wo"] = nc.dram_tensor("wo", [HV, DM], FP16, kind="ExternalInput").ap()
    dram["bq"] = nc.dram_tensor("bq", [HV], F32, kind="ExternalInput").ap()
    dram["bk"] = nc.dram_tensor("bk", [HV], F32, kind="ExternalInput").ap()
    dram["masks"] = nc.dram_tensor("masks", [128, 128], FP16, kind="ExternalInput").ap()
    dram["ident"] = nc.dram_tensor("ident", [128, 128], FP16, kind="ExternalInput").ap()
    o = nc.dram_tensor("o", [S, DM], F32, kind="ExternalOutput").ap()

    with tile.TileContext(nc) as tc:
        for _ in range(nbody):
            _build_kernel(tc, nc, dram, o, debug=nbody == -1)
    nc.compile()
    return nc


def _build_debug_nc():
    nc = bacc.Bacc("TRN2", target_bir_lowering=False, debug=False)
    dram = {}
    for nm in ("xh", "xl"):
        dram[nm] = nc.dram_tensor(nm, [DM, S], FP8, kind="ExternalInput").ap()
    for nm in ("wqh", "wql", "wkh", "wkl", "wvh", "wvl"):
        dram[nm] = nc.dram_tensor(nm, [DM, HV], FP8, kind="ExternalInput").ap()
    dram["wo"] = nc.dram_tensor("wo", [HV, DM], FP16, kind="ExternalInput").ap()
    dram["bq"] = nc.dram_tensor("bq", [HV], F32, kind="ExternalInput").ap()
    dram["bk"] = nc.dram_tensor("bk", [HV], F32, kind="ExternalInput").ap()
    dram["masks"] = nc.dram_tensor("masks", [128, 128], FP16, kind="ExternalInput").ap()
    dram["ident"] = nc.dram_tensor("ident", [128, 128], FP16, kind="ExternalInput").ap()
    o = nc.dram_tensor("o", [S, DM], F32, kind="ExternalOutput").ap()
    dbg = {
        "d_qt8": nc.dram_tensor("d_qt8", [64, 2, S], FP8, kind="ExternalOutput").ap(),
        "d_kt8": nc.dram_tensor("d_kt8", [64, 2, S], FP8, kind="ExternalOutput").ap(),
        "d_vpr": nc.dram_tensor("d_vpr", [128, 8 * 65], FP16, kind="ExternalOutput").ap(),
        "d_an": nc.dram_tensor("d_an", [128, HV], FP16, kind="ExternalOutput").ap(),
        "d_an5": nc.dram_tensor("d_an5", [128, HV], FP16, kind="ExternalOutput").ap(),
        "d_an15": nc.dram_tensor("d_an15", [128, HV], FP16, kind="ExternalOutput").ap(),
        "d_at": nc.dram_tensor("d_at", [128, S], FP16, kind="ExternalOutput").ap(),
    }
    with tile.TileContext(nc) as tc:
        _build_kernel(tc, nc, dram, o, debug=dbg)
    nc.compile()
    return nc


def _build_kernel(tc, nc, dram, o, debug=None):
    EXP = mybir.ActivationFunctionType.Exp
    MULT = mybir.AluOpType.mult
    ADD = mybir.AluOpType.add

    with ExitStack() as ctx:
        # ---- persistent tensors (live across phases) ----
        pp = ctx.enter_context(tc.tile_pool(name="persist", bufs=1))
        # packed fp8 Q^T/K^T: quad tile q holds heads 4q..4q+3 at base
        # partition (h%4)*32; free dims = (dk-half t, s).  Score matmuls pass
        # an explicit tile_position so base partition 96 is usable.
        qt8 = [pp.tile([128, 2, S], FP8, name=f"qt8_{q}", tag=f"qt8_{q}") for q in range(2)]
        kt8 = [pp.tile([128, 2, S], FP8, name=f"kt8_{q}", tag=f"kt8_{q}") for q in range(2)]
        vpr = [
            pp.tile([128, 8 * 65], FP16, name=f"vp{sc}", tag=f"vp{sc}")
            for sc in range(S // 128)
        ]
        at_nat = [
            pp.tile([128, HV], FP16, name=f"an{sc}", tag=f"an{sc}")
            for sc in range(S // 128)
        ]
        at_sb = [
            pp.tile([128, S], FP16, name=f"at{p}", tag=f"at{p}")
            for p in range(NPAIR)
        ]
        wo_sb = pp.tile([128, NPAIR * DM], FP16, name="wo_sb", tag="wo_sb")
        mask_sb = pp.tile([128, 128], FP16, name="mask_sb", tag="mask_sb")
        ident_sb = pp.tile([128, 128], FP16, name="ident_sb", tag="ident_sb")
        bq_sb = pp.tile([128, NPAIR], F32, name="bq_sb", tag="bq_sb")
        bk_sb = pp.tile([128, NPAIR], F32, name="bk_sb", tag="bk_sb")
        # zero row for the au-zeroing matmul (see b_head)
        z_sb = pp.tile([1, 4 * 65], FP16, name="z_sb", tag="z_sb")
        # e^(1/256) base for Pool-engine exp offload (pow(base, st) = exp(st/256))
        e256_sb = pp.tile([128, 1024], F32, name="e256_sb", tag="e256_sb")
        # weights: per tensor a [128, KC2 * 2 * 512] fp8 tile, chunk kc2 at
        # [:, kc2, t, :]
        pa = ctx.enter_context(tc.tile_pool(name="pa", bufs=1))
        w_sb = {
            nm: pa.tile([128, KC2, 2, HV], FP8, name=f"{nm}_sb", tag=f"{nm}_sb")
            for nm in ("wqh", "wql", "wkh", "wkl", "wvh", "wvl")
        }
        pax = ctx.enter_context(tc.tile_pool(name="pa_x", bufs=8))
        pb = ctx.enter_context(tc.tile_pool(name="pb", bufs=8))
        pbr = ctx.enter_context(tc.tile_pool(name="pb_r", bufs=4))
        pc_pool = ctx.enter_context(tc.tile_pool(name="pc", bufs=3))
        # PSUM: pj 2x1 + st 2x2 + au 2x1 = 8 banks.
        psum = ctx.enter_context(tc.tile_pool(name="psum", bufs=2, space="PSUM"))

        nc.gpsimd.memset(z_sb[:], 0.0)
        nc.gpsimd.memset(e256_sb[:], float(np.exp(1.0 / 256.0)))
        for sc in range(S // 128):
            nc.gpsimd.memset(vpr[sc][:], 1.0)

        SH = S // 2

        # ---------- DMA (few, large transfers: HWDGE dispatch is ~0.6us per
        # DMA, so consolidate each tensor-half into one descriptor) ----------
        xtile = {}

        def load_x_q(half, nt):
            """load one 512-column slice of x (hi+lo)."""
            s0 = half * SH + nt * 512
            for v in ("xh", "xl"):
                t = pax.tile([128, KC2, 2, 512], FP8, name=f"xt_{v}_{half}_{nt}", tag="xt")
                nc.sync.dma_start(
                    out=t[:],
                    in_=dram[v][:, s0 : s0 + 512].rearrange(
                        "(k t p) s -> p k t s", p=128, t=2
                    ),
                )
                xtile[(v, half, nt)] = t

        def load_x(half):
            for nt in range(2):
                load_x_q(half, nt)

        def xts(v, half, kc2, nt):
            return xtile[(v, half, nt)][:, kc2]

        def load_w(nm):
            nc.sync.dma_start(
                out=w_sb[nm][:],
                in_=dram[nm].rearrange("(k t p) c -> p k t c", p=128, t=2),
            )

        load_x_q(0, 0)
        for nm in ("wqh", "wql", "wkh", "wkl"):
            load_w(nm)
        nc.sync.dma_start(
            out=bq_sb[:], in_=dram["bq"].rearrange("(bl r) -> r bl", r=128)
        )
        nc.sync.dma_start(
            out=bk_sb[:], in_=dram["bk"].rearrange("(bl r) -> r bl", r=128)
        )
        load_w("wvh")
        load_w("wvl")
        load_x_q(0, 1)
        nc.sync.dma_start(out=mask_sb[:], in_=dram["masks"])
        nc.sync.dma_start(out=ident_sb[:], in_=dram["ident"])

        # compensated-fp8 product passes: (xh,Wh), (xh,Wl), (xl,Wh)
        COMB = (("xh", "h"), ("xh", "l"), ("xl", "h"))

        # ---------- phase A building blocks ----------
        def a_v_chunk(half, ss):
            """V' tile for s-chunk (half*8 + ss): out [128 s, 512 cols]."""
            sc = half * (SH // 128) + ss
            vp_ps = psum.tile([128, 512], F32, name=f"vps_{sc}", tag="pj")
            n = 0
            for kc2 in range(KC2):
                for xv, wv_ in COMB:
                    n += 1
                    nc.tensor.matmul(
                        vp_ps[:],
                        lhsT=xts(xv, half, kc2, ss // 4)[:, :, (ss % 4) * 128 : (ss % 4 + 1) * 128],
                        rhs=w_sb["wv" + wv_][:, kc2],
                        start=(n == 1),
                        stop=(n == 3 * KC2),
                        perf_mode=DR,
                    )
            if False:
                nc.scalar.activation(
                    vpr[sc][:].rearrange("p (h c) -> p h c", h=8)[:, :, 0:64],
                    vp_ps[:].rearrange("p (h c) -> p h c", h=8),
                    mybir.ActivationFunctionType.Copy,
                    scale=1.0 / 32.0,
                )
            else:
                nc.vector.tensor_scalar(
                    out=vpr[sc][:].rearrange("p (h c) -> p h c", h=8)[:, :, 0:64],
                    in0=vp_ps[:].rearrange("p (h c) -> p h c", h=8),
                    scalar1=1.0 / 32.0,
                    scalar2=None,
                    op0=MULT,
                )

        def a_qk_block(wch, dst, b_sb, s2, half, bl, nt):
            """One packed-fp8 projection block: psum [128 cols', 512 s] ->
            fp8 quad tile. bl = quad*2 + t."""
            quad, tt = divmod(bl, 2)
            s0 = half * SH
            qs = s0 + nt * 512
            ps = psum.tile([128, 512], F32, name=f"qk_{wch}_{bl}_{qs}", tag="pj")
            n = 0
            for kc2 in range(KC2):
                for xv, wv_ in COMB:
                    n += 1
                    nc.tensor.matmul(
                        ps[:],
                        lhsT=w_sb[wch + wv_][:, kc2, :, bl * 128 : (bl + 1) * 128],
                        rhs=xts(xv, half, kc2, nt),
                        start=(n == 1),
                        stop=(n == 3 * KC2),
                        perf_mode=DR,
                    )
            nc.vector.tensor_scalar(
                out=dst[quad][:, tt, qs : qs + 512],
                in0=ps[:],
                scalar1=b_sb[:, bl : bl + 1],
                scalar2=s2,
                op0=ADD,
                op1=MULT,
            )

        # ---------- phase B building block ----------
        def b_head(h, j):
            """Attention for head h, q-tile j (512 queries)."""
            quad, hq = divmod(h, 4)
            r32 = hq * 32
            nk = 4 * j + 4  # causal: k-chunks 0..nk-1
            # au: 4 q-subchunks side by side, each [128 q, 64 attn + 1 denom].
            # The 4 causal accumulation groups share one PSUM bank, and a
            # start=True matmul marks the WHOLE 2KB bank pending-zero (which
            # would wipe sibling groups' partial sums) — so zero the tile with
            # one spanning matmul and accumulate everything with start=False.
            au = psum.tile([128, 4 * 65], F32, name=f"au_{h}_{j}", tag="au")
            nc.tensor.matmul(
                au[:],
                lhsT=z_sb[0:1, 0:128],
                rhs=z_sb[0:1, :],
                start=True,
                stop=True,
                skip_group_check=True,
            )

            def mk_st(pc):
                vp = max(0, 128 * (2 * pc) - 512 * j)
                st = psum.tile([128, 1024], F32, name=f"st_{h}_{j}_{pc}", tag="st")
                for u in range(2):
                    kc = 2 * pc + u
                    nc.tensor.matmul(
                        st[:, u * 512 + vp : (u + 1) * 512],
                        lhsT=kt8[quad][r32 : r32 + 32, :, kc * 128 : (kc + 1) * 128],
                        rhs=qt8[quad][r32 : r32 + 32, :, j * 512 + vp : (j + 1) * 512],
                        start=True,
                        stop=True,
                        perf_mode=DR,
                        tile_position=(r32, 0),
                    )
                pt = pb.tile([128, 1024], FP16, name=f"pt_{h}_{j}_{pc}", tag="pt")
                st3 = st[:].rearrange("p (u c) -> p u c", u=2)
                pt3 = pt[:].rearrange("p (u c) -> p u c", u=2)
                nc.scalar.activation(
                    pt3[:, :, vp:512], st3[:, :, vp:512], EXP, scale=1.0 / 256.0
                )
                for u in range(2):
                    kc = 2 * pc + u
                    i = kc - 4 * j
                    if i >= 0:  # diagonal chunk: triangular 0/1 mask
                        c0 = u * 512 + 128 * i
                        nc.vector.tensor_tensor(
                            out=pt[:, c0 : c0 + 128],
                            in0=pt[:, c0 : c0 + 128],
                            in1=mask_sb[:, 0:128],
                            op=MULT,
                        )
                return pt

            def mk_av(pc, pt):
                for u in range(2):
                    kc = 2 * pc + u
                    for qs4 in range(4):
                        jq = 4 * j + qs4
                        if kc > jq:
                            continue
                        nc.tensor.matmul(
                            au[:, qs4 * 65 : qs4 * 65 + 65],
                            lhsT=pt[:, u * 512 + qs4 * 128 : u * 512 + (qs4 + 1) * 128],
                            rhs=vpr[kc][:, h * 65 : (h + 1) * 65],
                            start=False,
                            stop=(kc == jq),
                            skip_group_check=True,
                        )

            # software pipeline: issue st(pc+1) before AV(pc) so the PE keeps
            # running while ACT computes exp(pc).
            pts = {0: mk_st(0)}
            for pc in range(nk // 2):
                if pc + 1 < nk // 2:
                    pts[pc + 1] = mk_st(pc + 1)
                mk_av(pc, pts.pop(pc))

            # normalization: per-partition reciprocal of the 4 denominator
            # columns, then one fused multiply per q-subchunk.
            rcp = pbr.tile([128, 4], F32R, name=f"r_{h}_{j}", tag="r")
            with nc.allow_low_precision(
                reason="f32r output is bit-identical to f32 here"
            ):
                nc.vector.reciprocal(
                    out=rcp[:],
                    in_=au[:].rearrange("p (q c) -> p q c", c=65)[:, :, 64],
                )
            for qs4 in range(4):
                if False:
                    nc.scalar.activation(
                        at_nat[4 * j + qs4][:, h * 64 : (h + 1) * 64],
                        au[:, qs4 * 65 : qs4 * 65 + 64],
                        mybir.ActivationFunctionType.Copy,
                        scale=rcp[:, qs4 : qs4 + 1].bitcast(F32),
                    )
                else:
                    nc.vector.tensor_scalar(
                        out=at_nat[4 * j + qs4][:, h * 64 : (h + 1) * 64],
                        in0=au[:, qs4 * 65 : qs4 * 65 + 64],
                        scalar1=rcp[:, qs4 : qs4 + 1].bitcast(F32),
                        scalar2=None,
                        op0=MULT,
                    )

        # ---------- transpose + phase C building blocks ----------
        def t_block(j, p):
            """Transpose at_nat[4j..4j+3] columns of pair p into at_sb[p]."""
            tp = psum.tile([128, 512], F32, name=f"tp_{j}_{p}", tag="pj")
            for qs4 in range(4):
                nc.tensor.matmul(
                    tp[:, qs4 * 128 : (qs4 + 1) * 128],
                    lhsT=at_nat[4 * j + qs4][:, p * 128 : (p + 1) * 128],
                    rhs=ident_sb[:],
                    start=True,
                    stop=True,
                )
            if j == 3:  # past the last exp: ACT is idle
                nc.scalar.copy(out=at_sb[p][:, j * 512 : (j + 1) * 512], in_=tp[:])
            else:
                nc.vector.tensor_copy(
                    out=at_sb[p][:, j * 512 : (j + 1) * 512], in_=tp[:]
                )

        def c_chunk(sc):
            osb = pc_pool.tile([128, DM], F32, name=f"osb_{sc}", tag="osb")
            for m in range(DM // 512):
                op_ps = psum.tile([128, 512], F32, name=f"ops_{sc}_{m}", tag="pj")
                for p in range(NPAIR):
                    nc.tensor.matmul(
                        op_ps[:],
                        lhsT=at_sb[p][:, sc * 128 : (sc + 1) * 128],
                        rhs=wo_sb[:, p * DM + m * 512 : p * DM + (m + 1) * 512],
                        start=(p == 0),
                        stop=(p == NPAIR - 1),
                    )
                if sc >= 12:
                    nc.scalar.copy(out=osb[:, m * 512 : (m + 1) * 512], in_=op_ps[:])
                else:
                    nc.vector.tensor_copy(
                        out=osb[:, m * 512 : (m + 1) * 512], in_=op_ps[:]
                    )
            nc.sync.dma_start(out=o[sc * 128 : (sc + 1) * 128, :], in_=osb[:])

        # ---------- interleaved schedule ----------
        def qk(half, bl, nt):
            return [
                lambda: a_qk_block("wq", qt8, bq_sb, 1.0 / 16.0, half, bl, nt),
                lambda: a_qk_block("wk", kt8, bk_sb, 0.5, half, bl, nt),
            ]

        # Minimal A prefix for B(h=0..3, j=0): Q/K pair tiles 0/1 (psum
        # blocks bl=0,1 of half 0) and V' chunks 0-3.
        for f in qk(0, 0, 0) + qk(0, 1, 0):
            f()
        for ss in range(4):
            a_v_chunk(0, ss)

        load_x(1)
        # remaining A work spread across B j=0..2 respecting column needs
        a_j0 = (
            qk(0, 2, 0) + qk(0, 3, 0)
            + [lambda ss=ss: a_v_chunk(0, ss) for ss in range(4, 8)]
            + qk(0, 0, 1) + qk(0, 1, 1)
        )
        a_j1 = (
            qk(0, 2, 1) + qk(0, 3, 1)
            + [lambda ss=ss: a_v_chunk(1, ss) for ss in range(4)]
            + qk(1, 0, 0) + qk(1, 1, 0) + qk(1, 2, 0) + qk(1, 3, 0)
        )
        a_j2 = (
            [lambda ss=ss: a_v_chunk(1, ss) for ss in range(4, 8)]
            + qk(1, 0, 1) + qk(1, 1, 1) + qk(1, 2, 1) + qk(1, 3, 1)
        )

        for h in range(8):
            b_head(h, 0)
            a_j0.pop(0)()
            if h % 2 == 1:
                a_j0.pop(0)()
        for p in range(NPAIR):
            t_block(0, p)
        nc.sync.dma_start(
            out=wo_sb[:].rearrange("p (pair c) -> p pair c", pair=NPAIR),
            in_=dram["wo"].rearrange("(pair p) c -> p pair c", p=128),
        )
        for h in range(8):
            b_head(h, 1)
            a_j1.pop(0)()
            a_j1.pop(0)()
        for p in range(NPAIR):
            t_block(1, p)
        for h in range(8):
            b_head(h, 2)
            a_j2.pop(0)()
            if h % 2 == 1:
                a_j2.pop(0)()
            if h % 2 == 1 and h // 2 < 4:
                c_chunk(h // 2)
        for p in range(NPAIR):
            t_block(2, p)
        for h in range(8):
            b_head(h, 3)
            c_chunk(4 + h)
        for p in range(NPAIR):
            t_block(3, p)
        for sc in range(12, 16):
            c_chunk(sc)

        if debug:
            nc.sync.dma_start(out=debug["d_qt8"], in_=qt8[0][:])
            nc.sync.dma_start(out=debug["d_kt8"], in_=kt8[0][:])
            nc.sync.dma_start(out=debug["d_vpr"], in_=vpr[0][:])
            nc.sync.dma_start(out=debug["d_an"], in_=at_nat[0][:])
            nc.sync.dma_start(out=debug["d_an5"], in_=at_nat[5][:])
            nc.sync.dma_start(out=debug["d_an15"], in_=at_nat[15][:])
            nc.sync.dma_start(out=debug["d_at"], in_=at_sb[0][:])


def _masks_np():
    # tri[r, c] = 1 where k_local <= q_local (unmasked on the diagonal block)
    r = np.arange(128)[:, None]
    c = np.arange(128)[None, :]
    return (c >= r).astype(np.float16)


def _qk_perm():
    """Column permutation mapping packed index bl*128 + (h%4)*32 + p to the
    natural column h*64 + t*32 + p (bl = (h//4)*2 + t)."""
    perm = np.empty(HV, np.int64)
    for h in range(8):
        for t in range(2):
            for p in range(32):
                bl = (h // 4) * 2 + t
                perm[bl * 128 + (h % 4) * 32 + p] = h * 64 + t * 32 + p
    return perm


def _split8(a):
    import ml_dtypes

    hi = np.asarray(a, np.float32).astype(ml_dtypes.float8_e4m3)
    lo = (np.asarray(a, np.float32) - hi.astype(np.float32)).astype(
        ml_dtypes.float8_e4m3
    )
    return hi, lo


def make_in_maps(input, Wq, bq, Wk, bk, Wv, Wo):
    scale = np.float32(1.0 / np.sqrt(D_K))
    masks = _masks_np()
    ident = np.eye(128, dtype=np.float16)
    perm = _qk_perm()
    input = np.asarray(input, np.float32)
    in_maps = []
    for c in range(NCORES):
        b, g = divmod(c, 2)
        cols = slice(g * HV, (g + 1) * HV)
        xh, xl = _split8(input[b].T)
        wqh, wql = _split8(np.asarray(Wq, np.float32)[:, cols][:, perm] * (scale * 256))
        wkh, wkl = _split8(np.asarray(Wk, np.float32)[:, cols][:, perm] * 32)
        wvh, wvl = _split8(np.asarray(Wv, np.float32)[:, cols] * 32)
        in_maps.append(
            {
                "xh": np.ascontiguousarray(xh),
                "xl": np.ascontiguousarray(xl),
                "wqh": np.ascontiguousarray(wqh),
                "wql": np.ascontiguousarray(wql),
                "wkh": np.ascontiguousarray(wkh),
                "wkl": np.ascontiguousarray(wkl),
                "wvh": np.ascontiguousarray(wvh),
                "wvl": np.ascontiguousarray(wvl),
                "bq": np.ascontiguousarray(
                    np.asarray(bq, np.float32)[cols][perm] * (scale * 256)
                ),
                "bk": np.ascontiguousarray(np.asarray(bk, np.float32)[cols][perm] * 32),
                "wo": np.ascontiguousarray(
                    np.asarray(Wo, np.float32)[g * HV : (g + 1) * HV, :]
                ).astype(np.float16),
                "masks": masks,
                "ident": ident,
            }
        )
    return in_maps


def _numpy_fallback(input, attn_mask, Wq, bq, Wk, bk, Wv, bv, Wo, bo):
    """Host fallback for non-causal masks (should not trigger in practice)."""
    x = np.asarray(input, np.float32)
    mask = np.asarray(attn_mask)
    B, S_, _ = x.shape
    scale = np.float32(1.0 / np.sqrt(D_K))
    out = np.empty((B, S_, D_MODEL), np.float32)
    for b in range(B):
        q = (x[b] @ Wq + bq).reshape(S_, N_HEAD, D_K)
        k = (x[b] @ Wk + bk).reshape(S_, N_HEAD, D_K)
        v = (x[b] @ Wv + bv).reshape(S_, N_HEAD, D_V)
        attn = np.empty((S_, N_HEAD, D_V), np.float32)
        for h in range(N_HEAD):
            score = (q[:, h] @ k[:, h].T) * scale
            score = np.where(mask, -np.inf, score)
            score -= score.max(axis=-1, keepdims=True)
            p = np.exp(score)
            p /= p.sum(axis=-1, keepdims=True)
            attn[:, h] = p @ v[:, h]
        out[b] = attn.reshape(S_, N_HEAD * D_V) @ Wo + bo
    return out


_CACHED_RUNNER = None


def _make_runner(nc):
    """Build the shard_map-jitted PJRT executor once; reuse across calls."""
    import jax
    from jax.sharding import Mesh, PartitionSpec
    from jax.experimental.shard_map import shard_map
    from concourse import bass2jax

    bass2jax.install_neuronx_cc_hook()
    partition_name = nc.partition_id_tensor.name if nc.partition_id_tensor else None
    in_names, out_names, out_avals, zero_outs = [], [], [], []
    for alloc in nc.m.functions[0].allocations:
        if not isinstance(alloc, mybir.MemoryLocationSet):
            continue
        name = alloc.memorylocations[0].name
        if alloc.kind == "ExternalInput":
            if name != partition_name:
                in_names.append(name)
        elif alloc.kind == "ExternalOutput":
            out_names.append(name)
            shape = tuple(alloc.tensor_shape)
            dtype = mybir.dt.np(alloc.dtype)
            out_avals.append(jax.core.ShapedArray(shape, dtype))
            zero_outs.append(np.zeros(shape, dtype))
    n_params = len(in_names)
    n_outs = len(out_avals)
    all_in_names = list(in_names) + list(out_names)
    if partition_name is not None:
        all_in_names.append(partition_name)

    def _body(*args):
        operands = list(args)
        if partition_name is not None:
            operands.append(bass2jax.partition_id_tensor())
        outs = bass2jax._bass_exec_p.bind(
            *operands,
            out_avals=tuple(out_avals),
            in_names=tuple(all_in_names),
            out_names=tuple(out_names),
            lowering_input_output_aliases=(),
            sim_require_finite=True,
            sim_require_nnan=True,
            nc=nc,
        )
        return tuple(outs)

    devices = jax.devices()[:NCORES]
    mesh = Mesh(np.asarray(devices), ("core",))
    sharded = jax.jit(
        shard_map(
            _body,
            mesh=mesh,
            in_specs=(PartitionSpec("core"),) * (n_params + n_outs),
            out_specs=(PartitionSpec("core"),) * n_outs,
            check_rep=False,
        ),
        donate_argnums=tuple(range(n_params, n_params + n_outs)),
        keep_unused=True,
    )

    def run(in_maps):
        concat_in = [
            np.concatenate(
                [np.asarray(in_maps[c][nm]) for c in range(NCORES)], axis=0
            )
            for nm in in_names
        ]
        concat_zeros = [
            np.zeros((NCORES * z.shape[0], *z.shape[1:]), z.dtype) for z in zero_outs
        ]
        out_arrs = sharded(*concat_in, *concat_zeros)
        return [
            {
                nm: np.asarray(out_arrs[i]).reshape(NCORES, *out_avals[i].shape)[c]
                for i, nm in enumerate(out_names)
            }
            for c in range(NCORES)
        ]

    return run


def kernel(input, attn_mask, Wq, bq, Wk, bk, Wv, bv, Wo, bo):
    causal = np.triu(np.ones((SEQ, SEQ), bool), k=1)
    if not np.array_equal(np.asarray(attn_mask), causal):
        return _numpy_fallback(input, attn_mask, Wq, bq, Wk, bk, Wv, bv, Wo, bo)

    global _CACHED_NC, _CACHED_RUNNER
    if _CACHED_NC is None:
        _CACHED_NC = _build_nc()

    in_maps = make_in_maps(input, Wq, bq, Wk, bk, Wv, Wo)
    try:
        if _CACHED_RUNNER is None:
            _CACHED_RUNNER = _make_runner(_CACHED_NC)
        outs = _CACHED_RUNNER(in_maps)
    except Exception:
        # jit-caching fast path failed (e.g. jax version skew) — use the
        # stock executor.
        _CACHED_RUNNER = None
        outs = bass_utils.run_bass_kernel_spmd(
            _CACHED_NC, in_maps, core_ids=list(range(NCORES))
        ).results

    corr = (
        np.asarray(bv, np.float32) @ np.asarray(Wo, np.float32)
        + np.asarray(bo, np.float32)
    ).astype(np.float32)
    out = np.empty((BATCH, SEQ, D_MODEL), np.float32)
    for b in range(BATCH):
        out[b] = outs[2 * b]["o"] + outs[2 * b + 1]["o"] + corr[None, :]
    return out


# revision 47
# speedup vs baseline: 1.1251x; 1.0008x over previous
"""Multi-head causal attention (B=4, S=2048, D=1024, H=16, dk=dv=64) on 8 NeuronCores.

Sharding: core c -> (batch b = c//2, head-group g = c%2 of 8 heads).
Each core computes Q/K/V projections for its batch restricted to its 8 heads,
causal softmax attention, and a partial output projection with its 512 rows of
Wo.  The host sums the two partials per batch and adds the constant correction
bv @ Wo + bo (bv passes through attention linearly because softmax rows sum
to 1).

v4 highlights (per core):
  - Projections run as compensated-fp8 DoubleRow matmuls: host splits x^T and
    the (range-scaled) weights into fp8 hi+lo pairs; x@W ~ xh@Wh + xh@Wl +
    xl@Wh costs 3 DoubleRow passes = 0.75x the f32r cost (measured end-to-end
    error 0.1%).
  - Q^T/K^T are emitted directly in the DoubleRow-packed fp8 layout
    ([128 = 4 heads x 32 dk, 2 dk-halves, S]) by permuting W's columns on the
    host, so the score matmuls run fp8 DoubleRow at 0.5 cycles/row with no
    repacking.  Scale 16 per side, undone by the exp's 1/256 input scale.
  - exp on ACT (f32 PSUM scores -> fp16 P^T); causal-masked diagonal chunks
    multiplied by a triangular 0/1 mask on the DVE (fp16 4x mode).
  - AV runs in natural orientation (out [128 q, 65]) using all 128 PE
    partitions (2x fewer cycles than transposed) with a ones column for the
    softmax denominator; normalization is a per-partition reciprocal + one
    fused DVE tensor_scalar per head.
  - attn tiles are transposed for the output projection via identity-matmul
    on the PE (128 cycles each).
  - phases are emitted interleaved so ACT exp work overlaps the PE-heavy
    projections and output projection; score tiles are software-pipelined.
"""

import numpy as np
from contextlib import ExitStack

import concourse.bass as bass
import concourse.mybir as mybir
import concourse.tile as tile
from concourse import bacc, bass_utils

N_HEAD, D_MODEL, D_K, D_V = 16, 1024, 64, 64
BATCH, SEQ = 4, 2048
NCORES = 8
S = SEQ
DM = D_MODEL
HV = 8 * D_V          # 512 local head-value columns per core
KC2 = DM // 256       # 4 DoubleRow contraction chunks
NPAIR = 4             # local head pairs
NQT = S // 512        # 4 q-tiles
F32 = mybir.dt.float32
F32R = mybir.dt.float32r
FP16 = mybir.dt.float16
FP8 = mybir.dt.float8e4
DR = mybir.MatmulPerfMode.DoubleRow

_CACHED_NC = None


def _build_nc(nbody=1):
    nc = bacc.Bacc("TRN2", target_bir_lowering=False, debug=False)

    dram = {}
    for nm in ("xh", "xl"):
        dram[nm] = nc.dram_tensor(nm, [DM, S], FP8, kind="ExternalInput").ap()
    for nm in ("wqh", "wql", "wkh", "wkl", "wvh", "wvl"):
        dram[nm] = nc.dram_tensor(nm, [DM, HV], FP8, kind="ExternalInput").ap()
    dram["wo"] = nc.dram_tensor("wo", [HV, DM], FP16, kind="ExternalInput").ap()
    dram["bq"] = nc.dram_tensor("bq", [HV], F32, kind="ExternalInput").ap()
    dram["bk"] = nc.dram_tensor("bk", [HV], F32, kind="ExternalInput").ap()
    dram["masks"] = nc.dram_tensor("masks", [128, 128], FP16, kind="ExternalInput").ap()
    dram["ident"] = nc.dram_tensor("ident", [128, 128], FP16, kind="ExternalInput").ap()
    o = nc.dram_tensor("o", [S, DM], F32, kind="ExternalOutput").ap()

    with tile.TileContext(nc) as tc:
        for _ in range(nbody):
            _build_kernel(tc, nc, dram, o, debug=nbody == -1)
    nc.compile()
    return nc


def _build_debug_nc():
    nc = bacc.Bacc("TRN2", target_bir_lowering=False, debug=False)
    dram = {}
    for nm in ("xh", "xl"):
        dram[nm] = nc.dram_tensor(nm, [DM, S], FP8, kind="ExternalInput").ap()
    for nm in ("wqh", "wql", "wkh", "wkl", "wvh", "wvl"):
        dram[nm] = nc.dram_tensor(nm, [DM, HV], FP8, kind="ExternalInput").ap()
    dram["wo"] = nc.dram_tensor("wo", [HV, DM], FP16, kind="ExternalInput").ap()
    dram["bq"] = nc.dram_tensor("bq", [HV], F32, kind="ExternalInput").ap()
    dram["bk"] = nc.dram_tensor("bk", [HV], F32, kind="ExternalInput").ap()
    dram["masks"] = nc.dram_tensor("masks", [128, 128], FP16, kind="ExternalInput").ap()
    dram["ident"] = nc.dram_tensor("ident", [128, 128], FP16, kind="ExternalInput").ap()
    o = nc.dram_tensor("o", [S, DM], F32, kind="ExternalOutput").ap()
    dbg = {
        "d_qt8": nc.dram_tensor("d_qt8", [64, 2, S], FP8, kind="ExternalOutput").ap(),
        "d_kt8": nc.dram_tensor("d_kt8", [64, 2, S], FP8, kind="ExternalOutput").ap(),
        "d_vpr": nc.dram_tensor("d_vpr", [128, 8 * 65], FP16, kind="ExternalOutput").ap(),
        "d_an": nc.dram_tensor("d_an", [128, HV], FP16, kind="ExternalOutput").ap(),
        "d_an5": nc.dram_tensor("d_an5", [128, HV], FP16, kind="ExternalOutput").ap(),
        "d_an15": nc.dram_tensor("d_an15", [128, HV], FP16, kind="ExternalOutput").ap(),
        "d_at": nc.dram_tensor("d_at", [128, S], FP16, kind="ExternalOutput").ap(),
    }
    with tile.TileContext(nc) as tc:
        _build_kernel(tc, nc, dram, o, debug=dbg)
    nc.compile()
    return nc


def _build_kernel(tc, nc, dram, o, debug=None):
    EXP = mybir.ActivationFunctionType.Exp
    MULT = mybir.AluOpType.mult
    ADD = mybir.AluOpType.add

    with ExitStack() as ctx:
        # ---- persistent tensors (live across phases) ----
        pp = ctx.enter_context(tc.tile_pool(name="persist", bufs=1))
        # packed fp8 Q^T/K^T: quad tile q holds heads 4q..4q+3 at base
        # partition (h%4)*32; free dims = (dk-half t, s).  Score matmuls pass
        # an explicit tile_position so base partition 96 is usable.
        qt8 = [pp.tile([128, 2, S], FP8, name=f"qt8_{q}", tag=f"qt8_{q}") for q in range(2)]
        kt8 = [pp.tile([128, 2, S], FP8, name=f"kt8_{q}", tag=f"kt8_{q}") for q in range(2)]
        vpr = [
            pp.tile([128, 8 * 65], FP16, name=f"vp{sc}", tag=f"vp{sc}")
            for sc in range(S // 128)
        ]
        at_nat = [
            pp.tile([128, HV], FP16, name=f"an{sc}", tag=f"an{sc}")
            for sc in range(S // 128)
        ]
        at_sb = [
            pp.tile([128, S], FP16, name=f"at{p}", tag=f"at{p}")
            for p in range(NPAIR)
        ]
        wo_sb = pp.tile([128, NPAIR * DM], FP16, name="wo_sb", tag="wo_sb")
        mask_sb = pp.tile([128, 128], FP16, name="mask_sb", tag="mask_sb")
        ident_sb = pp.tile([128, 128], FP16, name="ident_sb", tag="ident_sb")
        bq_sb = pp.tile([128, NPAIR], F32, name="bq_sb", tag="bq_sb")
        bk_sb = pp.tile([128, NPAIR], F32, name="bk_sb", tag="bk_sb")
        # zero row for the au-zeroing matmul (see b_head)
        z_sb = pp.tile([1, 4 * 65], FP16, name="z_sb", tag="z_sb")
        # e^(1/256) base for Pool-engine exp offload (pow(base, st) = exp(st/256))
        e256_sb = pp.tile([128, 1024], F32, name="e256_sb", tag="e256_sb")
        # weights: per tensor a [128, KC2 * 2 * 512] fp8 tile, chunk kc2 at
        # [:, kc2, t, :]
        pa = ctx.enter_context(tc.tile_pool(name="pa", bufs=1))
        w_sb = {
            nm: pa.tile([128, KC2, 2, HV], FP8, name=f"{nm}_sb", tag=f"{nm}_sb")
            for nm in ("wqh", "wql", "wkh", "wkl", "wvh", "wvl")
        }
        pax = ctx.enter_context(tc.tile_pool(name="pa_x", bufs=8))
        pb = ctx.enter_context(tc.tile_pool(name="pb", bufs=8))
        pbr = ctx.enter_context(tc.tile_pool(name="pb_r", bufs=4))
        pc_pool = ctx.enter_context(tc.tile_pool(name="pc", bufs=3))
        # PSUM: pj 2x1 + st 2x2 + au 2x1 = 8 banks.
        psum = ctx.enter_context(tc.tile_pool(name="psum", bufs=2, space="PSUM"))

        nc.gpsimd.memset(z_sb[:], 0.0)
        nc.gpsimd.memset(e256_sb[:], float(np.exp(1.0 / 256.0)))
        for sc in range(S // 128):
            nc.gpsimd.memset(vpr[sc][:], 1.0)

        SH = S // 2

        # ---------- DMA (few, large transfers: HWDGE dispatch is ~0.6us per
        # DMA, so consolidate each tensor-half into one descriptor) ----------
        xtile = {}

        def load_x_q(half, nt):
            """load one 512-column slice of x (hi+lo)."""
            s0 = half * SH + nt * 512
            for v in ("xh", "xl"):
                t = pax.tile([128, KC2, 2, 512], FP8, name=f"xt_{v}_{half}_{nt}", tag="xt")
                nc.sync.dma_start(
                    out=t[:],
                    in_=dram[v][:, s0 : s0 + 512].rearrange(
                        "(k t p) s -> p k t s", p=128, t=2
                    ),
                )
                xtile[(v, half, nt)] = t

        def load_x(half):
            for nt in range(2):
                load_x_q(half, nt)

        def xts(v, half, kc2, nt):
            return xtile[(v, half, nt)][:, kc2]

        def load_w(nm):
            nc.sync.dma_start(
                out=w_sb[nm][:],
                in_=dram[nm].rearrange("(k t p) c -> p k t c", p=128, t=2),
            )

        load_x_q(0, 0)
        for nm in ("wqh", "wql", "wkh", "wkl"):
            load_w(nm)
        nc.sync.dma_start(
            out=bq_sb[:], in_=dram["bq"].rearrange("(bl r) -> r bl", r=128)
        )
        nc.sync.dma_start(
            out=bk_sb[:], in_=dram["bk"].rearrange("(bl r) -> r bl", r=128)
        )
        load_w("wvh")
        load_w("wvl")
        load_x_q(0, 1)
        nc.sync.dma_start(out=mask_sb[:], in_=dram["masks"])
        nc.sync.dma_start(out=ident_sb[:], in_=dram["ident"])

        # compensated-fp8 product passes: (xh,Wh), (xh,Wl), (xl,Wh)
        COMB = (("xh", "h"), ("xh", "l"), ("xl", "h"))

        # ---------- phase A building blocks ----------
        def a_v_chunk(half, ss):
            """V' tile for s-chunk (half*8 + ss): out [128 s, 512 cols]."""
            sc = half * (SH // 128) + ss
            vp_ps = psum.tile([128, 512], F32, name=f"vps_{sc}", tag="pj")
            n = 0
            for kc2 in range(KC2):
                for xv, wv_ in COMB:
                    n += 1
                    nc.tensor.matmul(
                        vp_ps[:],
                        lhsT=xts(xv, half, kc2, ss // 4)[:, :, (ss % 4) * 128 : (ss % 4 + 1) * 128],
                        rhs=w_sb["wv" + wv_][:, kc2],
                        start=(n == 1),
                        stop=(n == 3 * KC2),
                        perf_mode=DR,
                    )
            if False:
                nc.scalar.activation(
                    vpr[sc][:].rearrange("p (h c) -> p h c", h=8)[:, :, 0:64],
                    vp_ps[:].rearrange("p (h c) -> p h c", h=8),
                    mybir.ActivationFunctionType.Copy,
                    scale=1.0 / 32.0,
                )
            else:
                nc.vector.tensor_scalar(
                    out=vpr[sc][:].rearrange("p (h c) -> p h c", h=8)[:, :, 0:64],
                    in0=vp_ps[:].rearrange("p (h c) -> p h c", h=8),
                    scalar1=1.0 / 32.0,
                    scalar2=None,
                    op0=MULT,
                )

        def a_qk_block(wch, dst, b_sb, s2, half, bl, nt):
            """One packed-fp8 projection block: psum [128 cols', 512 s] ->
            fp8 quad tile. bl = quad*2 + t."""
            quad, tt = divmod(bl, 2)
            s0 = half * SH
            qs = s0 + nt * 512
            ps = psum.tile([128, 512], F32, name=f"qk_{wch}_{bl}_{qs}", tag="pj")
            n = 0
            for kc2 in range(KC2):
                for xv, wv_ in COMB:
                    n += 1
                    nc.tensor.matmul(
                        ps[:],
                        lhsT=w_sb[wch + wv_][:, kc2, :, bl * 128 : (bl + 1) * 128],
                        rhs=xts(xv, half, kc2, nt),
                        start=(n == 1),
                        stop=(n == 3 * KC2),
                        perf_mode=DR,
                    )
            nc.vector.tensor_scalar(
                out=dst[quad][:, tt, qs : qs + 512],
                in0=ps[:],
                scalar1=b_sb[:, bl : bl + 1],
                scalar2=s2,
                op0=ADD,
                op1=MULT,
            )

        # ---------- phase B building block ----------
        def b_head(h, j):
            """Attention for head h, q-tile j (512 queries)."""
            quad, hq = divmod(h, 4)
            r32 = hq * 32
            nk = 4 * j + 4  # causal: k-chunks 0..nk-1
            # au: 4 q-subchunks side by side, each [128 q, 64 attn + 1 denom].
            # The 4 causal accumulation groups share one PSUM bank, and a
            # start=True matmul marks the WHOLE 2KB bank pending-zero (which
            # would wipe sibling groups' partial sums) — so zero the tile with
            # one spanning matmul and accumulate everything with start=False.
            au = psum.tile([128, 4 * 65], F32, name=f"au_{h}_{j}", tag="au")
            nc.tensor.matmul(
                au[:],
                lhsT=z_sb[0:1, 0:128],
                rhs=z_sb[0:1, :],
                start=True,
                stop=True,
                skip_group_check=True,
            )

            def mk_st(pc):
                vp = max(0, 128 * (2 * pc) - 512 * j)
                st = psum.tile([128, 1024], F32, name=f"st_{h}_{j}_{pc}", tag="st")
                for u in range(2):
                    kc = 2 * pc + u
                    # per-chunk causal trim (vpu >= vp); the exp below still
                    # covers [vp:512], reading bounded stale PSUM in the
                    # causally-dead [vp:vpu) strip, which no AV consumes.
                    vpu = max(0, 128 * kc - 512 * j)
                    nc.tensor.matmul(
                        st[:, u * 512 + vpu : (u + 1) * 512],
                        lhsT=kt8[quad][r32 : r32 + 32, :, kc * 128 : (kc + 1) * 128],
                        rhs=qt8[quad][r32 : r32 + 32, :, j * 512 + vpu : (j + 1) * 512],
                        start=True,
                        stop=True,
                        perf_mode=DR,
                        tile_position=(r32, 0),
                    )
                pt = pb.tile([128, 1024], FP16, name=f"pt_{h}_{j}_{pc}", tag="pt")
                st3 = st[:].rearrange("p (u c) -> p u c", u=2)
                pt3 = pt[:].rearrange("p (u c) -> p u c", u=2)
                nc.scalar.activation(
                    pt3[:, :, vp:512], st3[:, :, vp:512], EXP, scale=1.0 / 256.0
                )
                for u in range(2):
                    kc = 2 * pc + u
                    i = kc - 4 * j
                    if i >= 0:  # diagonal chunk: triangular 0/1 mask
                        c0 = u * 512 + 128 * i
                        nc.vector.tensor_tensor(
                            out=pt[:, c0 : c0 + 128],
                            in0=pt[:, c0 : c0 + 128],
                            in1=mask_sb[:, 0:128],
                            op=MULT,
                        )
                return pt

            def mk_av(pc, pt):
                for u in range(2):
                    kc = 2 * pc + u
                    for qs4 in range(4):
                        jq = 4 * j + qs4
                        if kc > jq:
                            continue
                        nc.tensor.matmul(
                            au[:, qs4 * 65 : qs4 * 65 + 65],
                            lhsT=pt[:, u * 512 + qs4 * 128 : u * 512 + (qs4 + 1) * 128],
                            rhs=vpr[kc][:, h * 65 : (h + 1) * 65],
                            start=False,
                            stop=(kc == jq),
                            skip_group_check=True,
                        )

            # software pipeline: issue st(pc+1) before AV(pc) so the PE keeps
            # running while ACT computes exp(pc).
            pts = {0: mk_st(0)}
            for pc in range(nk // 2):
                if pc + 1 < nk // 2:
                    pts[pc + 1] = mk_st(pc + 1)
                mk_av(pc, pts.pop(pc))

            # normalization: per-partition reciprocal of the 4 denominator
            # columns, then one fused multiply per q-subchunk.
            rcp = pbr.tile([128, 4], F32R, name=f"r_{h}_{j}", tag="r")
            with nc.allow_low_precision(
                reason="f32r output is bit-identical to f32 here"
            ):
                nc.vector.reciprocal(
                    out=rcp[:],
                    in_=au[:].rearrange("p (q c) -> p q c", c=65)[:, :, 64],
                )
            for qs4 in range(4):
                if False:
                    nc.scalar.activation(
                        at_nat[4 * j + qs4][:, h * 64 : (h + 1) * 64],
                        au[:, qs4 * 65 : qs4 * 65 + 64],
                        mybir.ActivationFunctionType.Copy,
                        scale=rcp[:, qs4 : qs4 + 1].bitcast(F32),
                    )
                else:
                    nc.vector.tensor_scalar(
                        out=at_nat[4 * j + qs4][:, h * 64 : (h + 1) * 64],
                        in0=au[:, qs4 * 65 : qs4 * 65 + 64],
                        scalar1=rcp[:, qs4 : qs4 + 1].bitcast(F32),
                        scalar2=None,
                        op0=MULT,
                    )

        # ---------- transpose + phase C building blocks ----------
        def t_block(j, p):
            """Transpose at_nat[4j..4j+3] columns of pair p into at_sb[p]."""
            tp = psum.tile([128, 512], F32, name=f"tp_{j}_{p}", tag="pj")
            for qs4 in range(4):
                nc.tensor.matmul(
                    tp[:, qs4 * 128 : (qs4 + 1) * 128],
                    lhsT=at_nat[4 * j + qs4][:, p * 128 : (p + 1) * 128],
                    rhs=ident_sb[:],
                    start=True,
                    stop=True,
                )
            if j == 3:  # past the last exp: ACT is idle
                nc.scalar.copy(out=at_sb[p][:, j * 512 : (j + 1) * 512], in_=tp[:])
            else:
                nc.vector.tensor_copy(
                    out=at_sb[p][:, j * 512 : (j + 1) * 512], in_=tp[:]
                )

        def c_chunk(sc):
            osb = pc_pool.tile([128, DM], F32, name=f"osb_{sc}", tag="osb")
            for m in range(DM // 512):
                op_ps = psum.tile([128, 512], F32, name=f"ops_{sc}_{m}", tag="pj")
                for p in range(NPAIR):
                    nc.tensor.matmul(
                        op_ps[:],
                        lhsT=at_sb[p][:, sc * 128 : (sc + 1) * 128],
                        rhs=wo_sb[:, p * DM + m * 512 : p * DM + (m + 1) * 512],
                        start=(p == 0),
                        stop=(p == NPAIR - 1),
                    )
                if sc >= 12:
                    nc.scalar.copy(out=osb[:, m * 512 : (m + 1) * 512], in_=op_ps[:])
                else:
                    nc.vector.tensor_copy(
                        out=osb[:, m * 512 : (m + 1) * 512], in_=op_ps[:]
                    )
            nc.sync.dma_start(out=o[sc * 128 : (sc + 1) * 128, :], in_=osb[:])

        # ---------- interleaved schedule ----------
        def qk(half, bl, nt):
            return [
                lambda: a_qk_block("wq", qt8, bq_sb, 1.0 / 16.0, half, bl, nt),
                lambda: a_qk_block("wk", kt8, bk_sb, 0.5, half, bl, nt),
            ]

        # Minimal A prefix for B(h=0..3, j=0): Q/K pair tiles 0/1 (psum
        # blocks bl=0,1 of half 0) and V' chunks 0-3.
        for f in qk(0, 0, 0) + qk(0, 1, 0):
            f()
        for ss in range(4):
            a_v_chunk(0, ss)

        load_x(1)
        # remaining A work spread across B j=0..2 respecting column needs
        a_j0 = (
            qk(0, 2, 0) + qk(0, 3, 0)
            + [lambda ss=ss: a_v_chunk(0, ss) for ss in range(4, 8)]
            + qk(0, 0, 1) + qk(0, 1, 1)
        )
        a_j1 = (
            qk(0, 2, 1) + qk(0, 3, 1)
            + [lambda ss=ss: a_v_chunk(1, ss) for ss in range(4)]
            + qk(1, 0, 0) + qk(1, 1, 0) + qk(1, 2, 0) + qk(1, 3, 0)
        )
        a_j2 = (
            [lambda ss=ss: a_v_chunk(1, ss) for ss in range(4, 8)]
            + qk(1, 0, 1) + qk(1, 1, 1) + qk(1, 2, 1) + qk(1, 3, 1)
        )

        for h in range(8):
            b_head(h, 0)
            a_j0.pop(0)()
            if h % 2 == 1:
                a_j0.pop(0)()
        for p in range(NPAIR):
            t_block(0, p)
        nc.sync.dma_start(
            out=wo_sb[:].rearrange("p (pair c) -> p pair c", pair=NPAIR),
            in_=dram["wo"].rearrange("(pair p) c -> p pair c", p=128),
        )
        for h in range(8):
            b_head(h, 1)
            a_j1.pop(0)()
            a_j1.pop(0)()
        for p in range(NPAIR):
            t_block(1, p)
        for h in range(8):
            b_head(h, 2)
            a_j2.pop(0)()
            if h % 2 == 1:
                a_j2.pop(0)()
            if h % 2 == 1 and h // 2 < 4:
                c_chunk(h // 2)
        for p in range(NPAIR):
            t_block(2, p)
        for h in range(8):
            b_head(h, 3)
            c_chunk(4 + h)
        for p in range(NPAIR):
            t_block(3, p)
        for sc in range(12, 16):
            c_chunk(sc)

        if debug:
            nc.sync.dma_start(out=debug["d_qt8"], in_=qt8[0][:])
            nc.sync.dma_start(out=debug["d_kt8"], in_=kt8[0][:])
            nc.sync.dma_start(out=debug["d_vpr"], in_=vpr[0][:])
            nc.sync.dma_start(out=debug["d_an"], in_=at_nat[0][:])
            nc.sync.dma_start(out=debug["d_an5"], in_=at_nat[5][:])
            nc.sync.dma_start(out=debug["d_an15"], in_=at_nat[15][:])
            nc.sync.dma_start(out=debug["d_at"], in_=at_sb[0][:])


def _masks_np():
    # tri[r, c] = 1 where k_local <= q_local (unmasked on the diagonal block)
    r = np.arange(128)[:, None]
    c = np.arange(128)[None, :]
    return (c >= r).astype(np.float16)


def _qk_perm():
    """Column permutation mapping packed index bl*128 + (h%4)*32 + p to the
    natural column h*64 + t*32 + p (bl = (h//4)*2 + t)."""
    perm = np.empty(HV, np.int64)
    for h in range(8):
        for t in range(2):
            for p in range(32):
                bl = (h // 4) * 2 + t
                perm[bl * 128 + (h % 4) * 32 + p] = h * 64 + t * 32 + p
    return perm


def _split8(a):
    import ml_dtypes

    hi = np.asarray(a, np.float32).astype(ml_dtypes.float8_e4m3)
    lo = (np.asarray(a, np.float32) - hi.astype(np.float32)).astype(
        ml_dtypes.float8_e4m3
    )
    return hi, lo


def make_in_maps(input, Wq, bq, Wk, bk, Wv, Wo):
    scale = np.float32(1.0 / np.sqrt(D_K))
    masks = _masks_np()
    ident = np.eye(128, dtype=np.float16)
    perm = _qk_perm()
    input = np.asarray(input, np.float32)
    in_maps = []
    for c in range(NCORES):
        b, g = divmod(c, 2)
        cols = slice(g * HV, (g + 1) * HV)
        xh, xl = _split8(input[b].T)
        wqh, wql = _split8(np.asarray(Wq, np.float32)[:, cols][:, perm] * (scale * 256))
        wkh, wkl = _split8(np.asarray(Wk, np.float32)[:, cols][:, perm] * 32)
        wvh, wvl = _split8(np.asarray(Wv, np.float32)[:, cols] * 32)
        in_maps.append(
            {
                "xh": np.ascontiguousarray(xh),
                "xl": np.ascontiguousarray(xl),
                "wqh": np.ascontiguousarray(wqh),
                "wql": np.ascontiguousarray(wql),
                "wkh": np.ascontiguousarray(wkh),
                "wkl": np.ascontiguousarray(wkl),
                "wvh": np.ascontiguousarray(wvh),
                "wvl": np.ascontiguousarray(wvl),
                "bq": np.ascontiguousarray(
                    np.asarray(bq, np.float32)[cols][perm] * (scale * 256)
                ),
                "bk": np.ascontiguousarray(np.asarray(bk, np.float32)[cols][perm] * 32),
                "wo": np.ascontiguousarray(
                    np.asarray(Wo, np.float32)[g * HV : (g + 1) * HV, :]
                ).astype(np.float16),
                "masks": masks,
                "ident": ident,
            }
        )
    return in_maps


def _numpy_fallback(input, attn_mask, Wq, bq, Wk, bk, Wv, bv, Wo, bo):
    """Host fallback for non-causal masks (should not trigger in practice)."""
    x = np.asarray(input, np.float32)
    mask = np.asarray(attn_mask)
    B, S_, _ = x.shape
    scale = np.float32(1.0 / np.sqrt(D_K))
    out = np.empty((B, S_, D_MODEL), np.float32)
    for b in range(B):
        q = (x[b] @ Wq + bq).reshape(S_, N_HEAD, D_K)
        k = (x[b] @ Wk + bk).reshape(S_, N_HEAD, D_K)
        v = (x[b] @ Wv + bv).reshape(S_, N_HEAD, D_V)
        attn = np.empty((S_, N_HEAD, D_V), np.float32)
        for h in range(N_HEAD):
            score = (q[:, h] @ k[:, h].T) * scale
            score = np.where(mask, -np.inf, score)
            score -= score.max(axis=-1, keepdims=True)
            p = np.exp(score)
            p /= p.sum(axis=-1, keepdims=True)
            attn[:, h] = p @ v[:, h]
        out[b] = attn.reshape(S_, N_HEAD * D_V) @ Wo + bo
    return out


_CACHED_RUNNER = None


def _make_runner(nc):
    """Build the shard_map-jitted PJRT executor once; reuse across calls."""
    import jax
    from jax.sharding import Mesh, PartitionSpec
    from jax.experimental.shard_map import shard_map
    from concourse import bass2jax

    bass2jax.install_neuronx_cc_hook()
    partition_name = nc.partition_id_tensor.name if nc.partition_id_tensor else None
    in_names, out_names, out_avals, zero_outs = [], [], [], []
    for alloc in nc.m.functions[0].allocations:
        if not isinstance(alloc, mybir.MemoryLocationSet):
            continue
        name = alloc.memorylocations[0].name
        if alloc.kind == "ExternalInput":
            if name != partition_name:
                in_names.append(name)
        elif alloc.kind == "ExternalOutput":
            out_names.append(name)
            shape = tuple(alloc.tensor_shape)
            dtype = mybir.dt.np(alloc.dtype)
            out_avals.append(jax.core.ShapedArray(shape, dtype))
            zero_outs.append(np.zeros(shape, dtype))
    n_params = len(in_names)
    n_outs = len(out_avals)
    all_in_names = list(in_names) + list(out_names)
    if partition_name is not None:
        all_in_names.append(partition_name)

    def _body(*args):
        operands = list(args)
        if partition_name is not None:
            operands.append(bass2jax.partition_id_tensor())
        outs = bass2jax._bass_exec_p.bind(
            *operands,
            out_avals=tuple(out_avals),
            in_names=tuple(all_in_names),
            out_names=tuple(out_names),
            lowering_input_output_aliases=(),
            sim_require_finite=True,
            sim_require_nnan=True,
            nc=nc,
        )
        return tuple(outs)

    devices = jax.devices()[:NCORES]
    mesh = Mesh(np.asarray(devices), ("core",))
    sharded = jax.jit(
        shard_map(
            _body,
            mesh=mesh,
            in_specs=(PartitionSpec("core"),) * (n_params + n_outs),
            out_specs=(PartitionSpec("core"),) * n_outs,
            check_rep=False,
        ),
        donate_argnums=tuple(range(n_params, n_params + n_outs)),
        keep_unused=True,
    )

    def run(in_maps):
        concat_in = [
            np.concatenate(
                [np.asarray(in_maps[c][nm]) for c in range(NCORES)], axis=0
            )
            for nm in in_names
        ]
        concat_zeros = [
            np.zeros((NCORES * z.shape[0], *z.shape[1:]), z.dtype) for z in zero_outs
        ]
        out_arrs = sharded(*concat_in, *concat_zeros)
        return [
            {
                nm: np.asarray(out_arrs[i]).reshape(NCORES, *out_avals[i].shape)[c]
                for i, nm in enumerate(out_names)
            }
            for c in range(NCORES)
        ]

    return run


def kernel(input, attn_mask, Wq, bq, Wk, bk, Wv, bv, Wo, bo):
    causal = np.triu(np.ones((SEQ, SEQ), bool), k=1)
    if not np.array_equal(np.asarray(attn_mask), causal):
        return _numpy_fallback(input, attn_mask, Wq, bq, Wk, bk, Wv, bv, Wo, bo)

    global _CACHED_NC, _CACHED_RUNNER
    if _CACHED_NC is None:
        _CACHED_NC = _build_nc()

    in_maps = make_in_maps(input, Wq, bq, Wk, bk, Wv, Wo)
    try:
        if _CACHED_RUNNER is None:
            _CACHED_RUNNER = _make_runner(_CACHED_NC)
        outs = _CACHED_RUNNER(in_maps)
    except Exception:
        # jit-caching fast path failed (e.g. jax version skew) — use the
        # stock executor.
        _CACHED_RUNNER = None
        outs = bass_utils.run_bass_kernel_spmd(
            _CACHED_NC, in_maps, core_ids=list(range(NCORES))
        ).results

    corr = (
        np.asarray(bv, np.float32) @ np.asarray(Wo, np.float32)
        + np.asarray(bo, np.float32)
    ).astype(np.float32)
    out = np.empty((BATCH, SEQ, D_MODEL), np.float32)
    for b in range(BATCH):
        out[b] = outs[2 * b]["o"] + outs[2 * b + 1]["o"] + corr[None, :]
    return out
